# revision 1
# baseline (speedup 1.0000x reference)
"""Bass/Tile TRN2 kernel for nn_AttentionHead (B=64, N=1024, d=512), 8-core data parallel.

Math (per batch):
    proj  = x @ W1 + b1                      [N, 2d]
    S     = proj @ relu(proj).T / sqrt(2d)   [N, N]
    P     = softmax(S, axis=-1)
    F     = P @ proj                         [N, 2d]
    out   = relu(F @ W2 + b2)                [N, d]

Kernel dataflow (transposed-score formulation, avoids transposing P):
    xT    = x.T (DMA transpose)                                 [d, N]
    projT = W1.T @ xT + b1; keyT = relu(projT)                  [2d, N]
    St[m,n] = sum_e keyT[e,m] projT[e,n];  Et = exp(St / 32)    [m, n]
    r[n]  = sum_m Et[m,n]            (ones-column matmul)
    G[d,n] = sum_m x[m,d] Et[m,n]
    H[e,n] = sum_d W1[d,e] G[d,n]
          (= unnormalized P@(x@W1) transposed; b1's value-path contribution is
           r[n]*b1[e], folded into the fc2 bias row c below)
    Z[n,t] = sum_e H[e,n] W2[e,t] + r[n]*c[t],  c = b1 @ W2 + b2
    out   = relu(Z[n,t] / r[n])
All matmul operands bf16 (fp32 PSUM accumulate). Loops are ordered so each
stationary (lhsT) tile serves the two 512-wide free-dim chunks back-to-back;
a post-compile pass (_dedup_ldweights) elides the repeated LDWEIGHTS.
"""

import numpy as np

B, N, D = 64, 1024, 512
E = 2 * D
NCORES = 8
BPC = B // NCORES
P = 128
MG = N // P  # 8 token groups
DG = D // P  # 4 d groups
EG = E // P  # 8 e groups
NJ = N // 512  # 2 free-dim chunks
SCALE = float(1.0 / np.sqrt(2.0 * D))

_CACHE = {}
_PATCHED = False


def _dedup_ldweights(nc):
    """Delete redundant InstLdweights: consecutive PE weight-loads of the same
    SBUF region keep the PE array's stationary operand, so the repeat load is a
    no-op costing ~107ns. Only sync-free LDWs are removed (waits/updates were
    already hoisted by bacc's move_matmul_waits_to_ldweights)."""
    import concourse.mybir as mybir

    removed = 0
    for bb in nc.m.functions[0].blocks:
        last_key = None
        keep = []
        for inst in bb.instructions:
            if str(getattr(inst, "engine", "")) != "EngineType.PE":
                keep.append(inst)
                continue
            if isinstance(inst, mybir.InstLdweights):
                ap = inst.ins[0]
                key = (
                    getattr(ap, "memref", None),
                    getattr(ap, "offset", None),
                    str(getattr(ap, "ap", None)),
                    str(getattr(ap, "dtype", None)),
                    str(getattr(inst, "tile_position", None)),
                    str(getattr(inst, "is_transpose", None)),
                )
                si = inst.sync_info
                sync_free = si is None or (not si.on_wait and not si.on_update)
                if key == last_key and sync_free:
                    removed += 1
                    continue
                last_key = key
            keep.append(inst)
        bb.instructions[:] = keep
    return removed


def _build(bpc=BPC):
    import concourse.mybir as mybir
    import concourse.tile as tile
    from concourse import bacc
    from contextlib import ExitStack

    BF = mybir.dt.bfloat16
    F32 = mybir.dt.float32
    AF = mybir.ActivationFunctionType
    ALU = mybir.AluOpType

    nc = bacc.Bacc("TRN2", target_bir_lowering=False, debug=False, num_devices=NCORES)
    x_d = nc.dram_tensor("x", [bpc, N, D], F32, kind="ExternalInput").ap()
    w1_d = nc.dram_tensor("W1", [D, E], F32, kind="ExternalInput").ap()
    b1_d = nc.dram_tensor("bias1", [E], F32, kind="ExternalInput").ap()
    w2_d = nc.dram_tensor("W2", [E, D], F32, kind="ExternalInput").ap()
    b2_d = nc.dram_tensor("bias2", [D], F32, kind="ExternalInput").ap()
    c_d = nc.dram_tensor("c", [D], F32, kind="ExternalInput").ap()  # b1@W2 + b2 (host)
    out_d = nc.dram_tensor("out", [bpc, N, D], F32, kind="ExternalOutput").ap()

    with tile.TileContext(nc) as tc, ExitStack() as ctx:
        stage = ctx.enter_context(tc.tile_pool(name="stage", bufs=2))
        consts = ctx.enter_context(tc.tile_pool(name="consts", bufs=1))
        xbf_p = ctx.enter_context(tc.tile_pool(name="xbf", bufs=2))
        xt_p = ctx.enter_context(tc.tile_pool(name="xt", bufs=2))
        projT_p = ctx.enter_context(tc.tile_pool(name="projT", bufs=1))
        keyT_p = ctx.enter_context(tc.tile_pool(name="keyT", bufs=1))
        e_p = ctx.enter_context(tc.tile_pool(name="Et", bufs=1))
        g_p = ctx.enter_context(tc.tile_pool(name="Gt", bufs=1))
        h_p = ctx.enter_context(tc.tile_pool(name="Ht", bufs=1))
        outp = ctx.enter_context(tc.tile_pool(name="outp", bufs=2))
        small = ctx.enter_context(tc.tile_pool(name="small", bufs=2))
        dram = ctx.enter_context(tc.tile_pool(name="dram", bufs=2, space="DRAM"))
        ps = ctx.enter_context(tc.tile_pool(name="ps", bufs=6, space="PSUM"))
        psC = ctx.enter_context(tc.tile_pool(name="psC", bufs=2, space="PSUM"))

        # ---------------- first batch's x in flight before anything else ----
        # (per-m-group loads on the sync queue so cast+transpose can chase them;
        #  weights go on other queues in parallel)
        x_st = stage.tile([P, MG, D], F32, tag="stage")
        for mg in range(MG):
            nc.sync.dma_start(
                out=x_st[:, mg, :], in_=x_d[0][mg * P : (mg + 1) * P, :]
            )

        # ---------------- constants / weights ----------------
        w1_st = stage.tile([P, DG, E], F32, tag="stage")
        nc.scalar.dma_start(out=w1_st[:], in_=w1_d.rearrange("(dg p) e -> p dg e", p=P))
        w1_bf = consts.tile([P, DG, E], BF)
        nc.gpsimd.tensor_copy(w1_bf[:], w1_st[:])

        w2_st = stage.tile([P, EG, D], F32, tag="stage")
        nc.scalar.dma_start(out=w2_st[:], in_=w2_d.rearrange("(eg p) t -> p eg t", p=P))
        w2_bf = consts.tile([P, EG, D], BF)
        nc.gpsimd.tensor_copy(w2_bf[:], w2_st[:])

        b1t = consts.tile([P, EG], F32)
        nc.scalar.dma_start(out=b1t[:], in_=b1_d.rearrange("(g p) -> p g", p=P))
        ones_sq = consts.tile([P, P], BF)
        nc.vector.memset(ones_sq[:], 1.0)

        # c = b1@W2 + b2 (host-computed input), broadcast to all partitions:
        # the value-path bias contribution to fc2 is out += c[t] (post 1/r scale).
        import concourse.bass as bass_mod

        c_bcast = consts.tile([P, D], F32)
        c_src = c_d.rearrange("(o t) -> o t", o=1)
        c_bcast_ap = bass_mod.AP(
            tensor=c_src.tensor,
            offset=c_src.offset,
            ap=[[0, P], c_src.ap[1]],
        )
        nc.scalar.dma_start(out=c_bcast[:], in_=c_bcast_ap)

        # ---------------- per-batch pipeline ----------------
        for b in range(bpc):
            if b > 0:
                x_st = stage.tile([P, MG, D], F32, tag="stage")
                nc.sync.dma_start(
                    out=x_st[:], in_=x_d[b].rearrange("(g p) d -> p g d", p=P)
                )
            # per-m-group cast + xbar DMA transpose (SBUF -> SBUF) so the first
            # fc1 matmuls aren't gated on the whole batch being staged
            x_bf = xbf_p.tile([P, MG, D], BF)
            xT = xt_p.tile([P, DG, N], BF)
            for mg in range(MG):
                nc.vector.tensor_copy(x_bf[:, mg, :], x_st[:, mg, :])
                nc.sync.dma_start(
                    out=xT[:, :, mg * P : (mg + 1) * P],
                    in_=x_bf[:, mg, :],
                    transpose=True,
                )

            # fc1: projT = W1.T @ xT + b1 ; keyT = relu(projT)
            projT = projT_p.tile([P, EG, N], BF)
            keyT = keyT_p.tile([P, EG, N], BF)
            for eg in range(EG):
                pf = [ps.tile([P, 512], F32, tag="ps", name=f"pf{eg}_{j}") for j in range(NJ)]
                for dg in range(DG):
                    for nj in range(NJ):
                        nc.tensor.matmul(
                            pf[nj][:],
                            w1_bf[:, dg, eg * P : (eg + 1) * P],
                            xT[:, dg, nj * 512 : (nj + 1) * 512],
                            start=(dg == 0), stop=(dg == DG - 1),
                        )
                for nj in range(NJ):
                    nsl = slice(nj * 512, (nj + 1) * 512)
                    nc.scalar.activation(
                        projT[:, eg, nsl], pf[nj][:], AF.Identity,
                        bias=b1t[:, eg : eg + 1], scale=1.0,
                    )
                    nc.vector.tensor_scalar(
                        out=keyT[:, eg, nsl], in0=pf[nj][:],
                        scalar1=b1t[:, eg : eg + 1], scalar2=0.0,
                        op0=ALU.add, op1=ALU.max,
                    )

            # St[m,n] = sum_e keyT[e,m] * projT[e,n];  Et = exp(St/32)
            Et = e_p.tile([P, MG, N], BF)
            for mg in range(MG):
                pst = [ps.tile([P, 512], F32, tag="ps", name=f"pst{mg}_{j}") for j in range(NJ)]
                for eg in range(EG):
                    for nj in range(NJ):
                        nc.tensor.matmul(
                            pst[nj][:],
                            keyT[:, eg, mg * P : (mg + 1) * P],
                            projT[:, eg, nj * 512 : (nj + 1) * 512],
                            start=(eg == 0), stop=(eg == EG - 1),
                        )
                for nj in range(NJ):
                    nc.scalar.activation(
                        Et[:, mg, nj * 512 : (nj + 1) * 512], pst[nj][:], AF.Exp,
                        bias=0.0, scale=SCALE,
                    )

            # rowsum r[n] = sum_m Et[m,n] (all-ones stationary; any psum row = sum)
            r_f32 = small.tile([1, N], F32)
            pr = [ps.tile([P, 512], F32, tag="ps", name=f"pr{j}") for j in range(NJ)]
            for mg in range(MG):
                for nj in range(NJ):
                    nc.tensor.matmul(
                        pr[nj][:], ones_sq[:], Et[:, mg, nj * 512 : (nj + 1) * 512],
                        start=(mg == 0), stop=(mg == MG - 1),
                    )
            for nj in range(NJ):
                nsl = slice(nj * 512, (nj + 1) * 512)
                nc.vector.tensor_copy(r_f32[:, nsl], pr[nj][0:1, :])

            # G[d,n] = sum_m x[m,d] Et[m,n]
            Gt = g_p.tile([P, DG, N], BF)
            for dg in range(DG):
                pg = [ps.tile([P, 512], F32, tag="ps", name=f"pg{dg}_{j}") for j in range(NJ)]
                for mg in range(MG):
                    for nj in range(NJ):
                        nc.tensor.matmul(
                            pg[nj][:],
                            x_bf[:, mg, dg * P : (dg + 1) * P],
                            Et[:, mg, nj * 512 : (nj + 1) * 512],
                            start=(mg == 0), stop=(mg == MG - 1),
                        )
                for nj in range(NJ):
                    nc.vector.tensor_copy(
                        Gt[:, dg, nj * 512 : (nj + 1) * 512], pg[nj][:]
                    )

            # H[e,n] = sum_d W1[d,e] G[d,n]
            Ht = h_p.tile([P, EG, N], BF)
            for eg in range(EG):
                ph = [ps.tile([P, 512], F32, tag="ps", name=f"ph{eg}_{j}") for j in range(NJ)]
                for dg in range(DG):
                    for nj in range(NJ):
                        nc.tensor.matmul(
                            ph[nj][:],
                            w1_bf[:, dg, eg * P : (eg + 1) * P],
                            Gt[:, dg, nj * 512 : (nj + 1) * 512],
                            start=(dg == 0), stop=(dg == DG - 1),
                        )
                for nj in range(NJ):
                    nc.scalar.copy(Ht[:, eg, nj * 512 : (nj + 1) * 512], ph[nj][:])

            # 1/r in [n-partition, 1] layout (bounce through DRAM to transpose)
            r_dram = dram.tile([N], F32)
            nc.sync.dma_start(out=r_dram.rearrange("(o n) -> o n", o=1), in_=r_f32[:1, :])
            rT = small.tile([P, MG], F32)
            nc.sync.dma_start(out=rT[:], in_=r_dram.rearrange("(j p) -> p j", p=P))
            rinv = small.tile([P, MG], F32)
            nc.vector.reciprocal(rinv[:], rT[:])

            # fc2: Z[n,t] = sum_e H[e,n] W2[e,t];  out = relu(Z/r + c)
            o_t = outp.tile([P, MG, D], F32)
            for ng in range(MG):
                po = psC.tile([P, D], F32, tag="psC")
                for eg in range(EG):
                    nc.tensor.matmul(
                        po[:],
                        Ht[:, eg, ng * P : (ng + 1) * P],
                        w2_bf[:, eg, :],
                        start=(eg == 0), stop=(eg == EG - 1),
                    )
                osl = o_t[:, ng, :]
                nc.scalar.activation(
                    osl, po[:], AF.Copy, bias=0.0, scale=rinv[:, ng : ng + 1]
                )
                nc.vector.tensor_add(osl, osl, c_bcast[:])
                nc.vector.tensor_scalar_max(osl, osl, 0.0)
                if b == bpc - 1:
                    nc.sync.dma_start(
                        out=out_d[b][ng * P : (ng + 1) * P, :], in_=osl
                    )
            if b < bpc - 1:
                nc.sync.dma_start(
                    out=out_d[b].rearrange("(g p) t -> p g t", p=P), in_=o_t[:]
                )

    nc.compile()
    _dedup_ldweights(nc)
    return nc


def get_nc(bpc=BPC):
    if bpc not in _CACHE:
        _CACHE[bpc] = _build(bpc)
    return _CACHE[bpc]


def kernel(x, W1, bias1, W2, bias2):
    from concourse.bass_utils import run_bass_kernel_spmd

    nc = get_nc()
    x = np.ascontiguousarray(x, dtype=np.float32)
    W1 = np.asarray(W1, dtype=np.float32)
    bias1 = np.asarray(bias1, dtype=np.float32)
    W2 = np.asarray(W2, dtype=np.float32)
    bias2 = np.asarray(bias2, dtype=np.float32)
    c = (bias1 @ W2 + bias2).astype(np.float32)
    in_maps = [
        {
            "x": x[i * BPC : (i + 1) * BPC],
            "W1": W1,
            "bias1": bias1,
            "W2": W2,
            "bias2": bias2,
            "c": c,
        }
        for i in range(NCORES)
    ]
    res = run_bass_kernel_spmd(nc, in_maps, list(range(NCORES)))
    return np.concatenate([res.results[i]["out"] for i in range(NCORES)], axis=0)



# revision 23
# speedup vs baseline: 1.6792x; 1.6792x over previous
"""Bass/Tile TRN2 kernel for nn_AttentionHead (B=64, N=1024, d=512), 8-core data parallel.

Math (per batch):
    proj  = x @ W1 + b1                      [N, 2d]
    S     = proj @ relu(proj).T / sqrt(2d)   [N, N]
    P     = softmax(S, axis=-1)
    F     = P @ proj                         [N, 2d]
    out   = relu(F @ W2 + b2)                [N, d]

Kernel dataflow (transposed-score formulation + fc2 fusion):
    xT    = x.T (DMA transpose)                                 [d, N]
    projT = W1.T @ xT + b1; keyT = relu(projT)   (fp8/bf16)     [2d, N]
    St[m,n] = sum_e keyT[e,m] projT[e,n];  Et = exp(St / 32)    [m, n]
    r[n]  = sum_m Et[m,n]            (ones-column matmul)
    G[d,n] = sum_m x[m,d] Et[m,n]
    out[n,t] = relu( (sum_d G[d,n] W12[d,t]) / r[n] + c[t] )
  where W12 = W1 @ W2 and c = b1 @ W2 + b2 are host-precomputed: since
  P @ proj @ W2 = P@x@(W1 W2) + (P@1) b1 W2 and P rows sum to 1, the whole
  value-path fc1+fc2 collapses into a single [d,d] matmul vs [2d,*] twice.

The scores matmul runs in fp8-e4m3 DoubleRow (2 contraction tiles per
instruction, 2x PE throughput) for the first N8 e-group pairs and bf16 for the
rest, accumulating into the same PSUM bank; fp8 score error is damped by the
1/32 softmax temperature, sim-measured rel_l2 ~1.4e-2 at N8=4. All other
matmuls stay bf16 (value/output-path fp8 error does not average down).
Loops are ordered so each stationary (lhsT) tile serves its free-dim chunks
back-to-back; a post-compile pass (_dedup_ldweights) elides repeat LDWEIGHTS.
"""

import numpy as np

B, N, D = 64, 1024, 512
E = 2 * D
NCORES = 8
BPC = B // NCORES
P = 128
MG = N // P  # 8 token groups
DG = D // P  # 4 d groups
EG = E // P  # 8 e groups
NJ = N // 512  # 2 free-dim chunks
SCALE = float(1.0 / np.sqrt(2.0 * D))
N8 = 4  # e-group PAIRS of the scores contraction done in fp8 DoubleRow (0..4)

_CACHE = {}


def _dedup_ldweights(nc):
    """Delete redundant InstLdweights: consecutive PE weight-loads of the same
    SBUF region keep the PE array's stationary operand, so the repeat load is a
    no-op costing ~107ns. Only sync-free LDWs are removed (waits/updates were
    already hoisted by bacc's move_matmul_waits_to_ldweights)."""
    import concourse.mybir as mybir

    removed = 0
    for bb in nc.m.functions[0].blocks:
        last_key = None
        keep = []
        for inst in bb.instructions:
            if str(getattr(inst, "engine", "")) != "EngineType.PE":
                keep.append(inst)
                continue
            if isinstance(inst, mybir.InstLdweights):
                ap = inst.ins[0]
                key = (
                    getattr(ap, "memref", None),
                    getattr(ap, "offset", None),
                    str(getattr(ap, "ap", None)),
                    str(getattr(ap, "dtype", None)),
                    str(getattr(inst, "tile_position", None)),
                    str(getattr(inst, "is_transpose", None)),
                    str(getattr(inst, "perf_mode", None)),
                )
                si = inst.sync_info
                sync_free = si is None or (not si.on_wait and not si.on_update)
                if key == last_key and sync_free:
                    removed += 1
                    continue
                last_key = key
            keep.append(inst)
        bb.instructions[:] = keep
    return removed


def _build(bpc=BPC, n8=N8):
    import concourse.mybir as mybir
    import concourse.tile as tile
    from concourse import bacc
    from contextlib import ExitStack

    BF = mybir.dt.bfloat16
    F32 = mybir.dt.float32
    F8 = mybir.dt.float8e4
    AF = mybir.ActivationFunctionType
    ALU = mybir.AluOpType
    DR = mybir.MatmulPerfMode.DoubleRow

    e8 = 2 * n8  # e-groups handled in fp8
    nc = bacc.Bacc("TRN2", target_bir_lowering=False, debug=False, num_devices=NCORES)
    # x / W1 / W12 arrive host-pre-cast to bf16: halves their DMA traffic and
    # removes all on-device fp32->bf16 casts from the critical path.
    x_d = nc.dram_tensor("xbf", [bpc, N, D], BF, kind="ExternalInput").ap()
    xt_d = nc.dram_tensor("xTbf", [bpc, D, N], BF, kind="ExternalInput").ap()
    w1_d = nc.dram_tensor("W1bf", [D, E], BF, kind="ExternalInput").ap()
    b1_d = nc.dram_tensor("bias1", [E], F32, kind="ExternalInput").ap()
    w12_d = nc.dram_tensor("W12bf", [D, D], BF, kind="ExternalInput").ap()  # W1@W2
    c_d = nc.dram_tensor("c", [D], F32, kind="ExternalInput").ap()  # b1@W2 + b2
    out_d = nc.dram_tensor("out", [bpc, N, D], F32, kind="ExternalOutput").ap()

    with tile.TileContext(nc) as tc, ExitStack() as ctx:
        stage = ctx.enter_context(tc.tile_pool(name="stage", bufs=2))
        consts = ctx.enter_context(tc.tile_pool(name="consts", bufs=1))
        xt_p = ctx.enter_context(tc.tile_pool(name="xt", bufs=2))
        projT_p = ctx.enter_context(tc.tile_pool(name="projT", bufs=1))
        keyT_p = ctx.enter_context(tc.tile_pool(name="keyT", bufs=1))
        e_p = ctx.enter_context(tc.tile_pool(name="Et", bufs=1))
        g_p = ctx.enter_context(tc.tile_pool(name="Gt", bufs=1))
        outp = ctx.enter_context(tc.tile_pool(name="outp", bufs=2))
        small = ctx.enter_context(tc.tile_pool(name="small", bufs=2))
        dram = ctx.enter_context(tc.tile_pool(name="dram", bufs=2, space="DRAM"))
        ps = ctx.enter_context(tc.tile_pool(name="ps", bufs=6, space="PSUM"))
        psC = ctx.enter_context(tc.tile_pool(name="psC", bufs=2, space="PSUM"))

        # ---------------- first batch's x in flight before anything else ----
        # xT (host-pre-transposed) first: it gates fc1; per-dg chunks so the
        # first fc1 matmul can start after chunk 0 lands. Then x for G.
        xT = xt_p.tile([P, DG, N], BF, tag="xt")
        xt0_src = xt_d[0].rearrange("(dg p) n -> p dg n", p=P)
        for dg in range(DG):
            nc.sync.dma_start(out=xT[:, dg, :], in_=xt0_src[:, dg, :])
        x_st = stage.tile([P, MG, D], BF, tag="stage")
        nc.sync.dma_start(out=x_st[:], in_=x_d[0].rearrange("(g p) d -> p g d", p=P))

        # ---------------- constants / weights ----------------
        w1_bf = consts.tile([P, DG, E], BF)
        w1_src = w1_d.rearrange("(dg p) e -> p dg e", p=P)
        for dg in range(DG):
            nc.scalar.dma_start(out=w1_bf[:, dg, :], in_=w1_src[:, dg, :])

        w12_bf = consts.tile([P, DG, D], BF)
        nc.scalar.dma_start(
            out=w12_bf[:], in_=w12_d.rearrange("(dg p) t -> p dg t", p=P)
        )

        b1t = consts.tile([P, EG], F32)
        nc.gpsimd.dma_start(out=b1t[:], in_=b1_d.rearrange("(g p) -> p g", p=P))
        ones_sq = consts.tile([P, P], BF)
        nc.vector.memset(ones_sq[:], 1.0)

        # HAM warmup: keep the PE busy while the first batch stages so the
        # clock gate is already at 8/8 when the real matmul stream starts.
        wup = psC.tile([P, 256], F32, tag="psC", name="warmup")
        for _ in range(12):
            nc.tensor.matmul(wup[:, 0:P], ones_sq[:], ones_sq[:, 0:P], start=True, stop=True)
            nc.tensor.matmul(wup[:, P:256], ones_sq[:], ones_sq[:, 0:P], start=True, stop=True)

        # c = b1@W2 + b2 (host-computed input), broadcast to all partitions
        import concourse.bass as bass_mod

        c_bcast = consts.tile([P, D], F32)
        c_src = c_d.rearrange("(o t) -> o t", o=1)
        c_bcast_ap = bass_mod.AP(
            tensor=c_src.tensor,
            offset=c_src.offset,
            ap=[[0, P], c_src.ap[1]],
        )
        nc.gpsimd.dma_start(out=c_bcast[:], in_=c_bcast_ap)

        # ---------------- per-batch pipeline ----------------
        for b in range(bpc):
            if b > 0:
                xT = xt_p.tile([P, DG, N], BF, tag="xt")
                nc.sync.dma_start(
                    out=xT[:], in_=xt_d[b].rearrange("(dg p) n -> p dg n", p=P)
                )
                x_st = stage.tile([P, MG, D], BF, tag="stage")
                nc.sync.dma_start(
                    out=x_st[:], in_=x_d[b].rearrange("(g p) d -> p g d", p=P)
                )
            x_bf = x_st

            # fc1: projT = W1.T @ xT + b1 ; keyT = relu(projT)
            # first e8 e-groups are written fp8 (scores DoubleRow operands),
            # the rest bf16
            projT8 = projT_p.tile([P, e8, N], F8, name="projT8") if e8 else None
            keyT8 = keyT_p.tile([P, e8, N], F8, name="keyT8") if e8 else None
            nbf = EG - e8
            projTb = projT_p.tile([P, nbf, N], BF, name="projTb") if nbf else None
            keyTb = keyT_p.tile([P, nbf, N], BF, name="keyTb") if nbf else None
            for eg in range(EG):
                pf = [ps.tile([P, 512], F32, tag="ps", name=f"pf{eg}_{j}") for j in range(NJ)]
                for dg in range(DG):
                    for nj in range(NJ):
                        nc.tensor.matmul(
                            pf[nj][:],
                            w1_bf[:, dg, eg * P : (eg + 1) * P],
                            xT[:, dg, nj * 512 : (nj + 1) * 512],
                            start=(dg == 0), stop=(dg == DG - 1),
                        )
                if eg < e8:
                    pdst, kdst, ei = projT8, keyT8, eg
                else:
                    pdst, kdst, ei = projTb, keyTb, eg - e8
                for nj in range(NJ):
                    nsl = slice(nj * 512, (nj + 1) * 512)
                    nc.scalar.activation(
                        pdst[:, ei, nsl], pf[nj][:], AF.Identity,
                        bias=b1t[:, eg : eg + 1], scale=1.0,
                    )
                    nc.vector.tensor_scalar(
                        out=kdst[:, ei, nsl], in0=pf[nj][:],
                        scalar1=b1t[:, eg : eg + 1], scalar2=0.0,
                        op0=ALU.add, op1=ALU.max,
                    )


            # St[m,n] = sum_e keyT[e,m] * projT[e,n];  Et = exp(St/32)
            # fp8 e-group pairs via DoubleRow, remaining e-groups bf16, all
            # accumulating into the same PSUM tile.
            Et = e_p.tile([P, MG, N], BF)
            for mg in range(MG):
                pst = [ps.tile([P, 512], F32, tag="ps", name=f"pst{mg}_{j}") for j in range(NJ)]
                for egp in range(n8):
                    for nj in range(NJ):
                        nc.tensor.matmul(
                            pst[nj][:],
                            keyT8[:, 2 * egp : 2 * egp + 2, mg * P : (mg + 1) * P],
                            projT8[:, 2 * egp : 2 * egp + 2, nj * 512 : (nj + 1) * 512],
                            start=(egp == 0), stop=(egp == n8 - 1 and nbf == 0),
                            perf_mode=DR,
                        )
                for ei in range(nbf):
                    for nj in range(NJ):
                        nc.tensor.matmul(
                            pst[nj][:],
                            keyTb[:, ei, mg * P : (mg + 1) * P],
                            projTb[:, ei, nj * 512 : (nj + 1) * 512],
                            start=(n8 == 0 and ei == 0), stop=(ei == nbf - 1),
                        )
                for nj in range(NJ):
                    nc.scalar.activation(
                        Et[:, mg, nj * 512 : (nj + 1) * 512], pst[nj][:], AF.Exp,
                        bias=0.0, scale=SCALE,
                    )

            # rowsum r[n] = sum_m Et[m,n] (all-ones stationary; any psum row = sum)
            r_f32 = small.tile([1, N], F32)
            pr = [ps.tile([P, 512], F32, tag="ps", name=f"pr{j}") for j in range(NJ)]
            for mg in range(MG):
                for nj in range(NJ):
                    nc.tensor.matmul(
                        pr[nj][:], ones_sq[:], Et[:, mg, nj * 512 : (nj + 1) * 512],
                        start=(mg == 0), stop=(mg == MG - 1),
                    )
            for nj in range(NJ):
                nsl = slice(nj * 512, (nj + 1) * 512)
                nc.vector.tensor_copy(r_f32[:, nsl], pr[nj][0:1, :])

            # G[d,n] = sum_m x[m,d] Et[m,n]
            Gt = g_p.tile([P, DG, N], BF)
            for dg in range(DG):
                pg = [ps.tile([P, 512], F32, tag="ps", name=f"pg{dg}_{j}") for j in range(NJ)]
                for mg in range(MG):
                    for nj in range(NJ):
                        nc.tensor.matmul(
                            pg[nj][:],
                            x_bf[:, mg, dg * P : (dg + 1) * P],
                            Et[:, mg, nj * 512 : (nj + 1) * 512],
                            start=(mg == 0), stop=(mg == MG - 1),
                        )
                for nj in range(NJ):
                    nc.vector.tensor_copy(
                        Gt[:, dg, nj * 512 : (nj + 1) * 512], pg[nj][:]
                    )

            # 1/r in [n-partition, 1] layout (bounce through DRAM to transpose)
            r_dram = dram.tile([N], F32)
            nc.sync.dma_start(out=r_dram.rearrange("(o n) -> o n", o=1), in_=r_f32[:1, :])
            rT = small.tile([P, MG], F32)
            nc.sync.dma_start(out=rT[:], in_=r_dram.rearrange("(j p) -> p j", p=P))
            rinv = small.tile([P, MG], F32)
            nc.vector.reciprocal(rinv[:], rT[:])

            # fused fc2: Z[n,t] = sum_d G[d,n] W12[d,t];  out = relu(Z/r + c)
            o_t = outp.tile([P, MG, D], F32)
            for ng in range(MG):
                po = psC.tile([P, D], F32, tag="psC")
                for dg in range(DG):
                    nc.tensor.matmul(
                        po[:],
                        Gt[:, dg, ng * P : (ng + 1) * P],
                        w12_bf[:, dg, :],
                        start=(dg == 0), stop=(dg == DG - 1),
                    )
                if b == bpc - 1:
                    # split the epilogue into half-width chunks so the final
                    # ACT->add->relu->store chain pipelines (shorter tail);
                    # alternate store queues so descriptor issue isn't serial
                    for h in range(2):
                        hsl = slice(h * 256, (h + 1) * 256)
                        osl = o_t[:, ng, hsl]
                        nc.scalar.activation(
                            osl, po[:, hsl], AF.Copy, bias=0.0,
                            scale=rinv[:, ng : ng + 1],
                        )
                        nc.vector.tensor_add(osl, osl, c_bcast[:, hsl])
                        nc.vector.tensor_scalar_max(osl, osl, 0.0)
                        q = nc.gpsimd if h == 0 else nc.sync
                        q.dma_start(
                            out=out_d[b][ng * P : (ng + 1) * P, hsl], in_=osl
                        )
                else:
                    osl = o_t[:, ng, :]
                    nc.scalar.activation(
                        osl, po[:], AF.Copy, bias=0.0, scale=rinv[:, ng : ng + 1]
                    )
                    nc.vector.tensor_add(osl, osl, c_bcast[:])
                    nc.vector.tensor_scalar_max(osl, osl, 0.0)
            if b < bpc - 1:
                nc.gpsimd.dma_start(
                    out=out_d[b].rearrange("(g p) t -> p g t", p=P), in_=o_t[:]
                )

    nc.compile()
    _dedup_ldweights(nc)
    return nc


def get_nc(bpc=BPC, n8=N8):
    if (bpc, n8) not in _CACHE:
        _CACHE[(bpc, n8)] = _build(bpc, n8)
    return _CACHE[(bpc, n8)]


def make_in_maps(x, W1, bias1, W2, bias2):
    import ml_dtypes

    BF = ml_dtypes.bfloat16
    x = np.asarray(x, dtype=np.float32)
    W1 = np.asarray(W1, dtype=np.float32)
    bias1 = np.asarray(bias1, dtype=np.float32)
    W2 = np.asarray(W2, dtype=np.float32)
    bias2 = np.asarray(bias2, dtype=np.float32)
    xbf = np.ascontiguousarray(x.astype(BF))
    xTbf = np.ascontiguousarray(xbf.transpose(0, 2, 1))
    W1bf = np.ascontiguousarray(W1.astype(BF))
    W12bf = np.ascontiguousarray((W1 @ W2).astype(BF))
    c = (bias1 @ W2 + bias2).astype(np.float32)
    return [
        {
            "xbf": xbf[i * BPC : (i + 1) * BPC],
            "xTbf": xTbf[i * BPC : (i + 1) * BPC],
            "W1bf": W1bf,
            "bias1": bias1,
            "W12bf": W12bf,
            "c": c,
        }
        for i in range(NCORES)
    ]


def kernel(x, W1, bias1, W2, bias2):
    from concourse.bass_utils import run_bass_kernel_spmd

    nc = get_nc()
    in_maps = make_in_maps(x, W1, bias1, W2, bias2)
    res = run_bass_kernel_spmd(nc, in_maps, list(range(NCORES)))
    return np.concatenate([res.results[i]["out"] for i in range(NCORES)], axis=0)


# revision 24
# speedup vs baseline: 1.6819x; 1.0016x over previous
"""Bass/Tile TRN2 kernel for nn_AttentionHead (B=64, N=1024, d=512), 8-core data parallel.

Math (per batch):
    proj  = x @ W1 + b1                      [N, 2d]
    S     = proj @ relu(proj).T / sqrt(2d)   [N, N]
    P     = softmax(S, axis=-1)
    F     = P @ proj                         [N, 2d]
    out   = relu(F @ W2 + b2)                [N, d]

Kernel dataflow (transposed-score formulation + fc2 fusion):
    xT    = x.T (DMA transpose)                                 [d, N]
    projT = W1.T @ xT + b1; keyT = relu(projT)   (fp8/bf16)     [2d, N]
    St[m,n] = sum_e keyT[e,m] projT[e,n];  Et = exp(St / 32)    [m, n]
    r[n]  = sum_m Et[m,n]            (ones-column matmul)
    G[d,n] = sum_m x[m,d] Et[m,n]
    out[n,t] = relu( (sum_d G[d,n] W12[d,t]) / r[n] + c[t] )
  where W12 = W1 @ W2 and c = b1 @ W2 + b2 are host-precomputed: since
  P @ proj @ W2 = P@x@(W1 W2) + (P@1) b1 W2 and P rows sum to 1, the whole
  value-path fc1+fc2 collapses into a single [d,d] matmul vs [2d,*] twice.

The scores matmul runs in fp8-e4m3 DoubleRow (2 contraction tiles per
instruction, 2x PE throughput) for the first N8 e-group pairs and bf16 for the
rest, accumulating into the same PSUM bank; fp8 score error is damped by the
1/32 softmax temperature, sim-measured rel_l2 ~1.4e-2 at N8=4. All other
matmuls stay bf16 (value/output-path fp8 error does not average down).
Loops are ordered so each stationary (lhsT) tile serves its free-dim chunks
back-to-back; a post-compile pass (_dedup_ldweights) elides repeat LDWEIGHTS.
"""

import numpy as np

B, N, D = 64, 1024, 512
E = 2 * D
NCORES = 8
BPC = B // NCORES
P = 128
MG = N // P  # 8 token groups
DG = D // P  # 4 d groups
EG = E // P  # 8 e groups
NJ = N // 512  # 2 free-dim chunks
SCALE = float(1.0 / np.sqrt(2.0 * D))
N8 = 4  # e-group PAIRS of the scores contraction done in fp8 DoubleRow (0..4)

_CACHE = {}


def _dedup_ldweights(nc):
    """Delete redundant InstLdweights: consecutive PE weight-loads of the same
    SBUF region keep the PE array's stationary operand, so the repeat load is a
    no-op costing ~107ns. Only sync-free LDWs are removed (waits/updates were
    already hoisted by bacc's move_matmul_waits_to_ldweights)."""
    import concourse.mybir as mybir

    removed = 0
    for bb in nc.m.functions[0].blocks:
        last_key = None
        keep = []
        for inst in bb.instructions:
            if str(getattr(inst, "engine", "")) != "EngineType.PE":
                keep.append(inst)
                continue
            if isinstance(inst, mybir.InstLdweights):
                ap = inst.ins[0]
                key = (
                    getattr(ap, "memref", None),
                    getattr(ap, "offset", None),
                    str(getattr(ap, "ap", None)),
                    str(getattr(ap, "dtype", None)),
                    str(getattr(inst, "tile_position", None)),
                    str(getattr(inst, "is_transpose", None)),
                    str(getattr(inst, "perf_mode", None)),
                )
                si = inst.sync_info
                sync_free = si is None or (not si.on_wait and not si.on_update)
                if key == last_key and sync_free:
                    removed += 1
                    continue
                last_key = key
            keep.append(inst)
        bb.instructions[:] = keep
    return removed


def _build(bpc=BPC, n8=N8):
    import concourse.mybir as mybir
    import concourse.tile as tile
    from concourse import bacc
    from contextlib import ExitStack

    BF = mybir.dt.bfloat16
    F32 = mybir.dt.float32
    F8 = mybir.dt.float8e4
    AF = mybir.ActivationFunctionType
    ALU = mybir.AluOpType
    DR = mybir.MatmulPerfMode.DoubleRow

    e8 = 2 * n8  # e-groups handled in fp8
    nc = bacc.Bacc("TRN2", target_bir_lowering=False, debug=False, num_devices=NCORES)
    # x / W1 / W12 arrive host-pre-cast to bf16: halves their DMA traffic and
    # removes all on-device fp32->bf16 casts from the critical path.
    x_d = nc.dram_tensor("xbf", [bpc, N, D], BF, kind="ExternalInput").ap()
    xt_d = nc.dram_tensor("xTbf", [bpc, D, N], BF, kind="ExternalInput").ap()
    w1_d = nc.dram_tensor("W1bf", [D, E], BF, kind="ExternalInput").ap()
    b1_d = nc.dram_tensor("bias1", [E], F32, kind="ExternalInput").ap()
    w12_d = nc.dram_tensor("W12bf", [D, D], BF, kind="ExternalInput").ap()  # W1@W2
    c_d = nc.dram_tensor("c", [D], F32, kind="ExternalInput").ap()  # b1@W2 + b2
    out_d = nc.dram_tensor("out", [bpc, N, D], F32, kind="ExternalOutput").ap()

    with tile.TileContext(nc) as tc, ExitStack() as ctx:
        stage = ctx.enter_context(tc.tile_pool(name="stage", bufs=2))
        consts = ctx.enter_context(tc.tile_pool(name="consts", bufs=1))
        xt_p = ctx.enter_context(tc.tile_pool(name="xt", bufs=2))
        projT_p = ctx.enter_context(tc.tile_pool(name="projT", bufs=2))
        keyT_p = ctx.enter_context(tc.tile_pool(name="keyT", bufs=2))
        e_p = ctx.enter_context(tc.tile_pool(name="Et", bufs=2))
        g_p = ctx.enter_context(tc.tile_pool(name="Gt", bufs=2))
        outp = ctx.enter_context(tc.tile_pool(name="outp", bufs=2))
        small = ctx.enter_context(tc.tile_pool(name="small", bufs=2))
        dram = ctx.enter_context(tc.tile_pool(name="dram", bufs=2, space="DRAM"))
        ps = ctx.enter_context(tc.tile_pool(name="ps", bufs=6, space="PSUM"))
        psC = ctx.enter_context(tc.tile_pool(name="psC", bufs=2, space="PSUM"))

        # ---------------- first batch's x in flight before anything else ----
        # xT (host-pre-transposed) first: it gates fc1; per-dg chunks so the
        # first fc1 matmul can start after chunk 0 lands. Then x for G.
        xT = xt_p.tile([P, DG, N], BF, tag="xt")
        xt0_src = xt_d[0].rearrange("(dg p) n -> p dg n", p=P)
        for dg in range(DG):
            nc.sync.dma_start(out=xT[:, dg, :], in_=xt0_src[:, dg, :])
        x_st = stage.tile([P, MG, D], BF, tag="stage")
        nc.sync.dma_start(out=x_st[:], in_=x_d[0].rearrange("(g p) d -> p g d", p=P))

        # ---------------- constants / weights ----------------
        w1_bf = consts.tile([P, DG, E], BF)
        w1_src = w1_d.rearrange("(dg p) e -> p dg e", p=P)
        for dg in range(DG):
            nc.scalar.dma_start(out=w1_bf[:, dg, :], in_=w1_src[:, dg, :])

        w12_bf = consts.tile([P, DG, D], BF)
        nc.scalar.dma_start(
            out=w12_bf[:], in_=w12_d.rearrange("(dg p) t -> p dg t", p=P)
        )

        b1t = consts.tile([P, EG], F32)
        nc.gpsimd.dma_start(out=b1t[:], in_=b1_d.rearrange("(g p) -> p g", p=P))
        ones_sq = consts.tile([P, P], BF)
        nc.vector.memset(ones_sq[:], 1.0)

        # HAM warmup: keep the PE busy while the first batch stages so the
        # clock gate is already at 8/8 when the real matmul stream starts.
        wup = psC.tile([P, 256], F32, tag="psC", name="warmup")
        for _ in range(12):
            nc.tensor.matmul(wup[:, 0:P], ones_sq[:], ones_sq[:, 0:P], start=True, stop=True)
            nc.tensor.matmul(wup[:, P:256], ones_sq[:], ones_sq[:, 0:P], start=True, stop=True)

        # c = b1@W2 + b2 (host-computed input), broadcast to all partitions
        import concourse.bass as bass_mod

        c_bcast = consts.tile([P, D], F32)
        c_src = c_d.rearrange("(o t) -> o t", o=1)
        c_bcast_ap = bass_mod.AP(
            tensor=c_src.tensor,
            offset=c_src.offset,
            ap=[[0, P], c_src.ap[1]],
        )
        nc.gpsimd.dma_start(out=c_bcast[:], in_=c_bcast_ap)

        # ---------------- per-batch pipeline ----------------
        for b in range(bpc):
            if b > 0:
                xT = xt_p.tile([P, DG, N], BF, tag="xt")
                nc.sync.dma_start(
                    out=xT[:], in_=xt_d[b].rearrange("(dg p) n -> p dg n", p=P)
                )
                x_st = stage.tile([P, MG, D], BF, tag="stage")
                nc.sync.dma_start(
                    out=x_st[:], in_=x_d[b].rearrange("(g p) d -> p g d", p=P)
                )
            x_bf = x_st

            # fc1: projT = W1.T @ xT + b1 ; keyT = relu(projT)
            # first e8 e-groups are written fp8 (scores DoubleRow operands),
            # the rest bf16
            projT8 = projT_p.tile([P, e8, N], F8, name="projT8") if e8 else None
            keyT8 = keyT_p.tile([P, e8, N], F8, name="keyT8") if e8 else None
            nbf = EG - e8
            projTb = projT_p.tile([P, nbf, N], BF, name="projTb") if nbf else None
            keyTb = keyT_p.tile([P, nbf, N], BF, name="keyTb") if nbf else None
            for eg in range(EG):
                pf = [ps.tile([P, 512], F32, tag="ps", name=f"pf{eg}_{j}") for j in range(NJ)]
                for dg in range(DG):
                    for nj in range(NJ):
                        nc.tensor.matmul(
                            pf[nj][:],
                            w1_bf[:, dg, eg * P : (eg + 1) * P],
                            xT[:, dg, nj * 512 : (nj + 1) * 512],
                            start=(dg == 0), stop=(dg == DG - 1),
                        )
                if eg < e8:
                    pdst, kdst, ei = projT8, keyT8, eg
                else:
                    pdst, kdst, ei = projTb, keyTb, eg - e8
                for nj in range(NJ):
                    nsl = slice(nj * 512, (nj + 1) * 512)
                    nc.scalar.activation(
                        pdst[:, ei, nsl], pf[nj][:], AF.Identity,
                        bias=b1t[:, eg : eg + 1], scale=1.0,
                    )
                    nc.vector.tensor_scalar(
                        out=kdst[:, ei, nsl], in0=pf[nj][:],
                        scalar1=b1t[:, eg : eg + 1], scalar2=0.0,
                        op0=ALU.add, op1=ALU.max,
                    )


            # St[m,n] = sum_e keyT[e,m] * projT[e,n];  Et = exp(St/32)
            # fp8 e-group pairs via DoubleRow, remaining e-groups bf16, all
            # accumulating into the same PSUM tile.
            Et = e_p.tile([P, MG, N], BF)
            for mg in range(MG):
                pst = [ps.tile([P, 512], F32, tag="ps", name=f"pst{mg}_{j}") for j in range(NJ)]
                for egp in range(n8):
                    for nj in range(NJ):
                        nc.tensor.matmul(
                            pst[nj][:],
                            keyT8[:, 2 * egp : 2 * egp + 2, mg * P : (mg + 1) * P],
                            projT8[:, 2 * egp : 2 * egp + 2, nj * 512 : (nj + 1) * 512],
                            start=(egp == 0), stop=(egp == n8 - 1 and nbf == 0),
                            perf_mode=DR,
                        )
                for ei in range(nbf):
                    for nj in range(NJ):
                        nc.tensor.matmul(
                            pst[nj][:],
                            keyTb[:, ei, mg * P : (mg + 1) * P],
                            projTb[:, ei, nj * 512 : (nj + 1) * 512],
                            start=(n8 == 0 and ei == 0), stop=(ei == nbf - 1),
                        )
                for nj in range(NJ):
                    nc.scalar.activation(
                        Et[:, mg, nj * 512 : (nj + 1) * 512], pst[nj][:], AF.Exp,
                        bias=0.0, scale=SCALE,
                    )

            # rowsum r[n] = sum_m Et[m,n] (all-ones stationary; any psum row = sum)
            r_f32 = small.tile([1, N], F32)
            pr = [ps.tile([P, 512], F32, tag="ps", name=f"pr{j}") for j in range(NJ)]
            for mg in range(MG):
                for nj in range(NJ):
                    nc.tensor.matmul(
                        pr[nj][:], ones_sq[:], Et[:, mg, nj * 512 : (nj + 1) * 512],
                        start=(mg == 0), stop=(mg == MG - 1),
                    )
            for nj in range(NJ):
                nsl = slice(nj * 512, (nj + 1) * 512)
                nc.vector.tensor_copy(r_f32[:, nsl], pr[nj][0:1, :])

            # G[d,n] = sum_m x[m,d] Et[m,n]
            Gt = g_p.tile([P, DG, N], BF)
            for dg in range(DG):
                pg = [ps.tile([P, 512], F32, tag="ps", name=f"pg{dg}_{j}") for j in range(NJ)]
                for mg in range(MG):
                    for nj in range(NJ):
                        nc.tensor.matmul(
                            pg[nj][:],
                            x_bf[:, mg, dg * P : (dg + 1) * P],
                            Et[:, mg, nj * 512 : (nj + 1) * 512],
                            start=(mg == 0), stop=(mg == MG - 1),
                        )
                for nj in range(NJ):
                    nc.vector.tensor_copy(
                        Gt[:, dg, nj * 512 : (nj + 1) * 512], pg[nj][:]
                    )

            # 1/r in [n-partition, 1] layout (bounce through DRAM to transpose)
            r_dram = dram.tile([N], F32)
            nc.sync.dma_start(out=r_dram.rearrange("(o n) -> o n", o=1), in_=r_f32[:1, :])
            rT = small.tile([P, MG], F32)
            nc.sync.dma_start(out=rT[:], in_=r_dram.rearrange("(j p) -> p j", p=P))
            rinv = small.tile([P, MG], F32)
            nc.vector.reciprocal(rinv[:], rT[:])

            # fused fc2: Z[n,t] = sum_d G[d,n] W12[d,t];  out = relu(Z/r + c)
            o_t = outp.tile([P, MG, D], F32)
            for ng in range(MG):
                po = psC.tile([P, D], F32, tag="psC")
                for dg in range(DG):
                    nc.tensor.matmul(
                        po[:],
                        Gt[:, dg, ng * P : (ng + 1) * P],
                        w12_bf[:, dg, :],
                        start=(dg == 0), stop=(dg == DG - 1),
                    )
                if b == bpc - 1:
                    # split the epilogue into half-width chunks so the final
                    # ACT->add->relu->store chain pipelines (shorter tail);
                    # alternate store queues so descriptor issue isn't serial
                    for h in range(2):
                        hsl = slice(h * 256, (h + 1) * 256)
                        osl = o_t[:, ng, hsl]
                        nc.scalar.activation(
                            osl, po[:, hsl], AF.Copy, bias=0.0,
                            scale=rinv[:, ng : ng + 1],
                        )
                        nc.vector.tensor_add(osl, osl, c_bcast[:, hsl])
                        nc.vector.tensor_scalar_max(osl, osl, 0.0)
                        q = nc.gpsimd if h == 0 else nc.sync
                        q.dma_start(
                            out=out_d[b][ng * P : (ng + 1) * P, hsl], in_=osl
                        )
                else:
                    osl = o_t[:, ng, :]
                    nc.scalar.activation(
                        osl, po[:], AF.Copy, bias=0.0, scale=rinv[:, ng : ng + 1]
                    )
                    nc.vector.tensor_add(osl, osl, c_bcast[:])
                    nc.vector.tensor_scalar_max(osl, osl, 0.0)
            if b < bpc - 1:
                nc.gpsimd.dma_start(
                    out=out_d[b].rearrange("(g p) t -> p g t", p=P), in_=o_t[:]
                )

    nc.compile()
    _dedup_ldweights(nc)
    return nc


def get_nc(bpc=BPC, n8=N8):
    if (bpc, n8) not in _CACHE:
        _CACHE[(bpc, n8)] = _build(bpc, n8)
    return _CACHE[(bpc, n8)]


def make_in_maps(x, W1, bias1, W2, bias2):
    import ml_dtypes

    BF = ml_dtypes.bfloat16
    x = np.asarray(x, dtype=np.float32)
    W1 = np.asarray(W1, dtype=np.float32)
    bias1 = np.asarray(bias1, dtype=np.float32)
    W2 = np.asarray(W2, dtype=np.float32)
    bias2 = np.asarray(bias2, dtype=np.float32)
    xbf = np.ascontiguousarray(x.astype(BF))
    xTbf = np.ascontiguousarray(xbf.transpose(0, 2, 1))
    W1bf = np.ascontiguousarray(W1.astype(BF))
    W12bf = np.ascontiguousarray((W1 @ W2).astype(BF))
    c = (bias1 @ W2 + bias2).astype(np.float32)
    return [
        {
            "xbf": xbf[i * BPC : (i + 1) * BPC],
            "xTbf": xTbf[i * BPC : (i + 1) * BPC],
            "W1bf": W1bf,
            "bias1": bias1,
            "W12bf": W12bf,
            "c": c,
        }
        for i in range(NCORES)
    ]


def kernel(x, W1, bias1, W2, bias2):
    from concourse.bass_utils import run_bass_kernel_spmd

    nc = get_nc()
    in_maps = make_in_maps(x, W1, bias1, W2, bias2)
    res = run_bass_kernel_spmd(nc, in_maps, list(range(NCORES)))
    return np.concatenate([res.results[i]["out"] for i in range(NCORES)], axis=0)


# revision 36
# speedup vs baseline: 1.7754x; 1.0556x over previous
"""Bass/Tile TRN2 kernel for nn_AttentionHead (B=64, N=1024, d=512), 8-core data parallel.

Math (per batch):
    proj  = x @ W1 + b1                      [N, 2d]
    S     = proj @ relu(proj).T / sqrt(2d)   [N, N]
    P     = softmax(S, axis=-1)
    F     = P @ proj                         [N, 2d]
    out   = relu(F @ W2 + b2)                [N, d]

Kernel dataflow (transposed-score formulation + fc2 fusion):
    xT    = x.T (DMA transpose)                                 [d, N]
    projT = W1.T @ xT + b1; keyT = relu(projT)   (fp8/bf16)     [2d, N]
    St[m,n] = sum_e keyT[e,m] projT[e,n];  Et = exp(St / 32)    [m, n]
    r[n]  = sum_m Et[m,n]            (ones-column matmul)
    G[d,n] = sum_m x[m,d] Et[m,n]
    out[n,t] = relu( (sum_d G[d,n] W12[d,t]) / r[n] + c[t] )
  where W12 = W1 @ W2 and c = b1 @ W2 + b2 are host-precomputed: since
  P @ proj @ W2 = P@x@(W1 W2) + (P@1) b1 W2 and P rows sum to 1, the whole
  value-path fc1+fc2 collapses into a single [d,d] matmul vs [2d,*] twice.

The scores matmul runs in fp8-e4m3 DoubleRow (2 contraction tiles per
instruction, 2x PE throughput) for the first N8 e-group pairs and bf16 for the
rest, accumulating into the same PSUM bank; fp8 score error is damped by the
1/32 softmax temperature, sim-measured rel_l2 ~1.4e-2 at N8=4. All other
matmuls stay bf16 (value/output-path fp8 error does not average down).
Loops are ordered so each stationary (lhsT) tile serves its free-dim chunks
back-to-back; a post-compile pass (_dedup_ldweights) elides repeat LDWEIGHTS.
"""

import numpy as np

B, N, D = 64, 1024, 512
E = 2 * D
NCORES = 8
BPC = B // NCORES
P = 128
MG = N // P  # 8 token groups
DG = D // P  # 4 d groups
EG = E // P  # 8 e groups
NJ = N // 512  # 2 free-dim chunks
SCALE = float(1.0 / np.sqrt(2.0 * D))
N8 = 4  # e-group PAIRS of the scores contraction done in fp8 DoubleRow (0..4)
DITHER = 3.0 / 64.0  # projT pre-cast scale (1+a): decorrelates its e4m3
# rounding from keyT's (same values post-relu), halving the correlated
# error on the dominant S diagonal; exactly compensated in the exp scale.

_CACHE = {}


def _dedup_ldweights(nc):
    """Delete redundant InstLdweights: consecutive PE weight-loads of the same
    SBUF region keep the PE array's stationary operand, so the repeat load is a
    no-op costing ~107ns. Only sync-free LDWs are removed (waits/updates were
    already hoisted by bacc's move_matmul_waits_to_ldweights)."""
    import concourse.mybir as mybir

    removed = 0
    for bb in nc.m.functions[0].blocks:
        last_key = None
        keep = []
        for inst in bb.instructions:
            if str(getattr(inst, "engine", "")) != "EngineType.PE":
                keep.append(inst)
                continue
            if isinstance(inst, mybir.InstLdweights):
                ap = inst.ins[0]
                key = (
                    getattr(ap, "memref", None),
                    getattr(ap, "offset", None),
                    str(getattr(ap, "ap", None)),
                    str(getattr(ap, "dtype", None)),
                    str(getattr(inst, "tile_position", None)),
                    str(getattr(inst, "is_transpose", None)),
                    str(getattr(inst, "perf_mode", None)),
                )
                si = inst.sync_info
                sync_free = si is None or (not si.on_wait and not si.on_update)
                if key == last_key and sync_free:
                    removed += 1
                    continue
                last_key = key
            keep.append(inst)
        bb.instructions[:] = keep
    return removed


def _build(bpc=BPC, n8=N8):
    import concourse.mybir as mybir
    import concourse.tile as tile
    from concourse import bacc
    from contextlib import ExitStack

    BF = mybir.dt.bfloat16
    F32 = mybir.dt.float32
    F8 = mybir.dt.float8e4
    AF = mybir.ActivationFunctionType
    ALU = mybir.AluOpType
    DR = mybir.MatmulPerfMode.DoubleRow

    e8 = 2 * n8  # e-groups handled in fp8
    nc = bacc.Bacc("TRN2", target_bir_lowering=False, debug=False, num_devices=NCORES)
    # x / W1 / W12 arrive host-pre-cast to bf16: halves their DMA traffic and
    # removes all on-device fp32->bf16 casts from the critical path.
    # fc1 contraction split: d rows 0:256 via fp8 DoubleRow (host-cast
    # xT8 = e4m3(x.T/4), W18 = e4m3(4*W1) so the product scale is 1 and the
    # fp8 and bf16 halves accumulate into the same PSUM), rows 256:512 bf16.
    x_d = nc.dram_tensor("xbf", [bpc, N, D], BF, kind="ExternalInput").ap()
    xt8_d = nc.dram_tensor("xT8", [bpc, D // 2, N], F8, kind="ExternalInput").ap()
    xt_d = nc.dram_tensor("xTbf", [bpc, D // 2, N], BF, kind="ExternalInput").ap()
    w18_d = nc.dram_tensor("W18", [D // 2, E], F8, kind="ExternalInput").ap()
    w1_d = nc.dram_tensor("W1bf", [D // 2, E], BF, kind="ExternalInput").ap()
    b1_d = nc.dram_tensor("bias1", [E], F32, kind="ExternalInput").ap()
    b1a_d = nc.dram_tensor("bias1a", [E], F32, kind="ExternalInput").ap()  # (1+a)*b1
    w12_d = nc.dram_tensor("W12bf", [D, D], BF, kind="ExternalInput").ap()  # W1@W2
    c_d = nc.dram_tensor("c", [D], F32, kind="ExternalInput").ap()  # b1@W2 + b2
    out_d = nc.dram_tensor("out", [bpc, N, D], F32, kind="ExternalOutput").ap()

    with tile.TileContext(nc) as tc, ExitStack() as ctx:
        stage = ctx.enter_context(tc.tile_pool(name="stage", bufs=2))
        consts = ctx.enter_context(tc.tile_pool(name="consts", bufs=1))
        xt_p = ctx.enter_context(tc.tile_pool(name="xt", bufs=2))
        projT_p = ctx.enter_context(tc.tile_pool(name="projT", bufs=2))
        keyT_p = ctx.enter_context(tc.tile_pool(name="keyT", bufs=2))
        e_p = ctx.enter_context(tc.tile_pool(name="Et", bufs=2))
        g_p = ctx.enter_context(tc.tile_pool(name="Gt", bufs=2))
        outp = ctx.enter_context(tc.tile_pool(name="outp", bufs=2))
        small = ctx.enter_context(tc.tile_pool(name="small", bufs=2))
        dram = ctx.enter_context(tc.tile_pool(name="dram", bufs=2, space="DRAM"))
        ps = ctx.enter_context(tc.tile_pool(name="ps", bufs=6, space="PSUM"))
        psC = ctx.enter_context(tc.tile_pool(name="psC", bufs=2, space="PSUM"))

        # ---------------- first batch's x in flight before anything else ----
        # xT (host-pre-transposed, fp8 lo-half + bf16 hi-half) first: it gates
        # fc1; then x for the G stage.
        xT8 = xt_p.tile([P, 2, N], F8, tag="xt8")
        nc.sync.dma_start(out=xT8[:], in_=xt8_d[0].rearrange("(g p) n -> p g n", p=P))
        xT = xt_p.tile([P, 2, N], BF, tag="xt")
        nc.sync.dma_start(out=xT[:], in_=xt_d[0].rearrange("(g p) n -> p g n", p=P))
        # x (G-stage operand, not needed until ~30us in) goes on the scalar
        # queue so the sync queue serves only the fc1-gating xT loads
        x_st = stage.tile([P, MG, D], BF, tag="stage")
        nc.scalar.dma_start(out=x_st[:], in_=x_d[0].rearrange("(g p) d -> p g d", p=P))

        # ---------------- constants / weights ----------------
        w18 = consts.tile([P, 2, E], F8)
        nc.scalar.dma_start(out=w18[:], in_=w18_d.rearrange("(g p) e -> p g e", p=P))
        w1_bf = consts.tile([P, 2, E], BF)
        w1_src = w1_d.rearrange("(g p) e -> p g e", p=P)
        for dg in range(2):
            nc.scalar.dma_start(out=w1_bf[:, dg, :], in_=w1_src[:, dg, :])

        w12_bf = consts.tile([P, DG, D], BF)
        nc.scalar.dma_start(
            out=w12_bf[:], in_=w12_d.rearrange("(dg p) t -> p dg t", p=P)
        )

        b1t = consts.tile([P, EG], F32)
        nc.gpsimd.dma_start(out=b1t[:], in_=b1_d.rearrange("(g p) -> p g", p=P))
        b1ta = consts.tile([P, EG], F32)
        nc.gpsimd.dma_start(out=b1ta[:], in_=b1a_d.rearrange("(g p) -> p g", p=P))
        ones_sq = consts.tile([P, P], BF)
        nc.vector.memset(ones_sq[:], 1.0)

        # HAM warmup: keep the PE busy while the first batch stages so the
        # clock gate is already at 8/8 when the real matmul stream starts.
        wup = psC.tile([P, 256], F32, tag="psC", name="warmup")
        for _ in range(12):
            nc.tensor.matmul(wup[:, 0:P], ones_sq[:], ones_sq[:, 0:P], start=True, stop=True)
            nc.tensor.matmul(wup[:, P:256], ones_sq[:], ones_sq[:, 0:P], start=True, stop=True)

        # c = b1@W2 + b2 (host-computed input), broadcast to all partitions
        import concourse.bass as bass_mod

        c_bcast = consts.tile([P, D], F32)
        c_src = c_d.rearrange("(o t) -> o t", o=1)
        c_bcast_ap = bass_mod.AP(
            tensor=c_src.tensor,
            offset=c_src.offset,
            ap=[[0, P], c_src.ap[1]],
        )
        nc.gpsimd.dma_start(out=c_bcast[:], in_=c_bcast_ap)

        # ---------------- per-batch pipeline ----------------
        for b in range(bpc):
            if b > 0:
                xT8 = xt_p.tile([P, 2, N], F8, tag="xt8")
                nc.sync.dma_start(
                    out=xT8[:], in_=xt8_d[b].rearrange("(g p) n -> p g n", p=P)
                )
                xT = xt_p.tile([P, 2, N], BF, tag="xt")
                nc.sync.dma_start(
                    out=xT[:], in_=xt_d[b].rearrange("(g p) n -> p g n", p=P)
                )
                x_st = stage.tile([P, MG, D], BF, tag="stage")
                nc.sync.dma_start(
                    out=x_st[:], in_=x_d[b].rearrange("(g p) d -> p g d", p=P)
                )
            x_bf = x_st

            # fc1: projT = W1.T @ xT + b1 ; keyT = relu(projT)
            # first e8 e-groups are written fp8 (scores DoubleRow operands),
            # the rest bf16
            projT8 = projT_p.tile([P, e8, N], F8, name="projT8") if e8 else None
            keyT8 = keyT_p.tile([P, e8, N], F8, name="keyT8") if e8 else None
            nbf = EG - e8
            projTb = projT_p.tile([P, nbf, N], BF, name="projTb") if nbf else None
            keyTb = keyT_p.tile([P, nbf, N], BF, name="keyTb") if nbf else None
            for eg in range(EG):
                pf = [ps.tile([P, 512], F32, tag="ps", name=f"pf{eg}_{j}") for j in range(NJ)]
                for nj in range(NJ):
                    nc.tensor.matmul(
                        pf[nj][:],
                        w18[:, 0:2, eg * P : (eg + 1) * P],
                        xT8[:, 0:2, nj * 512 : (nj + 1) * 512],
                        start=True, stop=False,
                        perf_mode=DR,
                    )
                for dg in range(2):
                    for nj in range(NJ):
                        nc.tensor.matmul(
                            pf[nj][:],
                            w1_bf[:, dg, eg * P : (eg + 1) * P],
                            xT[:, dg, nj * 512 : (nj + 1) * 512],
                            start=False, stop=(dg == 1),
                        )
                if eg < e8:
                    pdst, kdst, ei = projT8, keyT8, eg
                else:
                    pdst, kdst, ei = projTb, keyTb, eg - e8
                for nj in range(NJ):
                    nsl = slice(nj * 512, (nj + 1) * 512)
                    nc.scalar.activation(
                        pdst[:, ei, nsl], pf[nj][:], AF.Identity,
                        bias=b1ta[:, eg : eg + 1], scale=1.0 + DITHER,
                    )
                    nc.vector.tensor_scalar(
                        out=kdst[:, ei, nsl], in0=pf[nj][:],
                        scalar1=b1t[:, eg : eg + 1], scalar2=0.0,
                        op0=ALU.add, op1=ALU.max,
                    )


            # St[m,n] = sum_e keyT[e,m] * projT[e,n];  Et = exp(St/32)
            # fp8 e-group pairs via DoubleRow, remaining e-groups bf16, all
            # accumulating into the same PSUM tile.
            Et = e_p.tile([P, MG, N], BF)
            for mg in range(MG):
                pst = [ps.tile([P, 512], F32, tag="ps", name=f"pst{mg}_{j}") for j in range(NJ)]
                for egp in range(n8):
                    for nj in range(NJ):
                        nc.tensor.matmul(
                            pst[nj][:],
                            keyT8[:, 2 * egp : 2 * egp + 2, mg * P : (mg + 1) * P],
                            projT8[:, 2 * egp : 2 * egp + 2, nj * 512 : (nj + 1) * 512],
                            start=(egp == 0), stop=(egp == n8 - 1 and nbf == 0),
                            perf_mode=DR,
                        )
                for ei in range(nbf):
                    for nj in range(NJ):
                        nc.tensor.matmul(
                            pst[nj][:],
                            keyTb[:, ei, mg * P : (mg + 1) * P],
                            projTb[:, ei, nj * 512 : (nj + 1) * 512],
                            start=(n8 == 0 and ei == 0), stop=(ei == nbf - 1),
                        )
                for nj in range(NJ):
                    nc.scalar.activation(
                        Et[:, mg, nj * 512 : (nj + 1) * 512], pst[nj][:], AF.Exp,
                        bias=0.0, scale=SCALE / (1.0 + DITHER),
                    )

            # rowsum r[n] = sum_m Et[m,n] (all-ones stationary; any psum row = sum)
            r_f32 = small.tile([1, N], F32)
            pr = [ps.tile([P, 512], F32, tag="ps", name=f"pr{j}") for j in range(NJ)]
            for mg in range(MG):
                for nj in range(NJ):
                    nc.tensor.matmul(
                        pr[nj][:], ones_sq[:], Et[:, mg, nj * 512 : (nj + 1) * 512],
                        start=(mg == 0), stop=(mg == MG - 1),
                    )
            for nj in range(NJ):
                nsl = slice(nj * 512, (nj + 1) * 512)
                nc.vector.tensor_copy(r_f32[:, nsl], pr[nj][0:1, :])

            # G[d,n] = sum_m x[m,d] Et[m,n]
            Gt = g_p.tile([P, DG, N], BF)
            for dg in range(DG):
                pg = [ps.tile([P, 512], F32, tag="ps", name=f"pg{dg}_{j}") for j in range(NJ)]
                for mg in range(MG):
                    for nj in range(NJ):
                        nc.tensor.matmul(
                            pg[nj][:],
                            x_bf[:, mg, dg * P : (dg + 1) * P],
                            Et[:, mg, nj * 512 : (nj + 1) * 512],
                            start=(mg == 0), stop=(mg == MG - 1),
                        )
                for nj in range(NJ):
                    nc.vector.tensor_copy(
                        Gt[:, dg, nj * 512 : (nj + 1) * 512], pg[nj][:]
                    )

            # 1/r in [n-partition, 1] layout (bounce through DRAM to transpose)
            r_dram = dram.tile([N], F32)
            nc.sync.dma_start(out=r_dram.rearrange("(o n) -> o n", o=1), in_=r_f32[:1, :])
            rT = small.tile([P, MG], F32)
            nc.sync.dma_start(out=rT[:], in_=r_dram.rearrange("(j p) -> p j", p=P))
            rinv = small.tile([P, MG], F32)
            nc.vector.reciprocal(rinv[:], rT[:])

            # fused fc2: Z[n,t] = sum_d G[d,n] W12[d,t];  out = relu(Z/r + c)
            o_t = outp.tile([P, MG, D], F32)
            for ng in range(MG):
                po = psC.tile([P, D], F32, tag="psC")
                for dg in range(DG):
                    nc.tensor.matmul(
                        po[:],
                        Gt[:, dg, ng * P : (ng + 1) * P],
                        w12_bf[:, dg, :],
                        start=(dg == 0), stop=(dg == DG - 1),
                    )
                if b == bpc - 1:
                    # split the epilogue into half-width chunks so the final
                    # ACT->add->relu->store chain pipelines (shorter tail);
                    # alternate store queues so descriptor issue isn't serial
                    for h in range(2):
                        hsl = slice(h * 256, (h + 1) * 256)
                        osl = o_t[:, ng, hsl]
                        nc.scalar.activation(
                            osl, po[:, hsl], AF.Copy, bias=0.0,
                            scale=rinv[:, ng : ng + 1],
                        )
                        nc.vector.tensor_add(osl, osl, c_bcast[:, hsl])
                        nc.vector.tensor_scalar_max(osl, osl, 0.0)
                        q = [nc.gpsimd, nc.sync, nc.scalar][(2 * ng + h) % 3]
                        q.dma_start(
                            out=out_d[b][ng * P : (ng + 1) * P, hsl], in_=osl
                        )
                else:
                    osl = o_t[:, ng, :]
                    nc.scalar.activation(
                        osl, po[:], AF.Copy, bias=0.0, scale=rinv[:, ng : ng + 1]
                    )
                    nc.vector.tensor_add(osl, osl, c_bcast[:])
                    nc.vector.tensor_scalar_max(osl, osl, 0.0)
            if b < bpc - 1:
                nc.gpsimd.dma_start(
                    out=out_d[b].rearrange("(g p) t -> p g t", p=P), in_=o_t[:]
                )

    nc.compile()
    _dedup_ldweights(nc)
    return nc


def get_nc(bpc=BPC, n8=N8):
    if (bpc, n8) not in _CACHE:
        _CACHE[(bpc, n8)] = _build(bpc, n8)
    return _CACHE[(bpc, n8)]


def make_in_maps(x, W1, bias1, W2, bias2):
    import ml_dtypes

    BF = ml_dtypes.bfloat16
    x = np.asarray(x, dtype=np.float32)
    W1 = np.asarray(W1, dtype=np.float32)
    bias1 = np.asarray(bias1, dtype=np.float32)
    W2 = np.asarray(W2, dtype=np.float32)
    bias2 = np.asarray(bias2, dtype=np.float32)
    E4 = ml_dtypes.float8_e4m3
    xbf = np.ascontiguousarray(x.astype(BF))
    xT = x.transpose(0, 2, 1)
    xT8 = np.ascontiguousarray((xT[:, : D // 2, :] / 4.0).astype(E4))
    xTbf = np.ascontiguousarray(xT[:, D // 2 :, :].astype(BF))
    W18 = np.ascontiguousarray((4.0 * W1[: D // 2, :]).astype(E4))
    W1bf = np.ascontiguousarray(W1[D // 2 :, :].astype(BF))
    W12bf = np.ascontiguousarray((W1 @ W2).astype(BF))
    c = (bias1 @ W2 + bias2).astype(np.float32)
    b1a = ((1.0 + DITHER) * bias1).astype(np.float32)
    return [
        {
            "xbf": xbf[i * BPC : (i + 1) * BPC],
            "xT8": xT8[i * BPC : (i + 1) * BPC],
            "xTbf": xTbf[i * BPC : (i + 1) * BPC],
            "W18": W18,
            "W1bf": W1bf,
            "bias1": bias1,
            "bias1a": b1a,
            "W12bf": W12bf,
            "c": c,
        }
        for i in range(NCORES)
    ]


def kernel(x, W1, bias1, W2, bias2):
    from concourse.bass_utils import run_bass_kernel_spmd

    nc = get_nc()
    in_maps = make_in_maps(x, W1, bias1, W2, bias2)
    res = run_bass_kernel_spmd(nc, in_maps, list(range(NCORES)))
    return np.concatenate([res.results[i]["out"] for i in range(NCORES)], axis=0)


# revision 38
# speedup vs baseline: 1.7929x; 1.0099x over previous
"""Bass/Tile TRN2 kernel for nn_AttentionHead (B=64, N=1024, d=512), 8-core data parallel.

Math (per batch):
    proj  = x @ W1 + b1                      [N, 2d]
    S     = proj @ relu(proj).T / sqrt(2d)   [N, N]
    P     = softmax(S, axis=-1)
    F     = P @ proj                         [N, 2d]
    out   = relu(F @ W2 + b2)                [N, d]

Kernel dataflow (transposed-score formulation + fc2 fusion):
    xT    = x.T (DMA transpose)                                 [d, N]
    projT = W1.T @ xT + b1; keyT = relu(projT)   (fp8/bf16)     [2d, N]
    St[m,n] = sum_e keyT[e,m] projT[e,n];  Et = exp(St / 32)    [m, n]
    r[n]  = sum_m Et[m,n]            (ones-column matmul)
    G[d,n] = sum_m x[m,d] Et[m,n]
    out[n,t] = relu( (sum_d G[d,n] W12[d,t]) / r[n] + c[t] )
  where W12 = W1 @ W2 and c = b1 @ W2 + b2 are host-precomputed: since
  P @ proj @ W2 = P@x@(W1 W2) + (P@1) b1 W2 and P rows sum to 1, the whole
  value-path fc1+fc2 collapses into a single [d,d] matmul vs [2d,*] twice.

The scores matmul runs in fp8-e4m3 DoubleRow (2 contraction tiles per
instruction, 2x PE throughput) for the first N8 e-group pairs and bf16 for the
rest, accumulating into the same PSUM bank; fp8 score error is damped by the
1/32 softmax temperature, sim-measured rel_l2 ~1.4e-2 at N8=4. All other
matmuls stay bf16 (value/output-path fp8 error does not average down).
Loops are ordered so each stationary (lhsT) tile serves its free-dim chunks
back-to-back; a post-compile pass (_dedup_ldweights) elides repeat LDWEIGHTS.
"""

import numpy as np

B, N, D = 64, 1024, 512
E = 2 * D
NCORES = 8
BPC = B // NCORES
P = 128
MG = N // P  # 8 token groups
DG = D // P  # 4 d groups
EG = E // P  # 8 e groups
NJ = N // 512  # 2 free-dim chunks
SCALE = float(1.0 / np.sqrt(2.0 * D))
N8 = 4  # e-group PAIRS of the scores contraction done in fp8 DoubleRow (0..4)
DITHER = 3.0 / 64.0  # projT pre-cast scale (1+a): decorrelates its e4m3
# rounding from keyT's (same values post-relu), halving the correlated
# error on the dominant S diagonal; exactly compensated in the exp scale.

_CACHE = {}


def _dedup_ldweights(nc):
    """Delete redundant InstLdweights: consecutive PE weight-loads of the same
    SBUF region keep the PE array's stationary operand, so the repeat load is a
    no-op costing ~107ns. Only sync-free LDWs are removed (waits/updates were
    already hoisted by bacc's move_matmul_waits_to_ldweights)."""
    import concourse.mybir as mybir

    removed = 0
    for bb in nc.m.functions[0].blocks:
        last_key = None
        keep = []
        for inst in bb.instructions:
            if str(getattr(inst, "engine", "")) != "EngineType.PE":
                keep.append(inst)
                continue
            if isinstance(inst, mybir.InstLdweights):
                ap = inst.ins[0]
                key = (
                    getattr(ap, "memref", None),
                    getattr(ap, "offset", None),
                    str(getattr(ap, "ap", None)),
                    str(getattr(ap, "dtype", None)),
                    str(getattr(inst, "tile_position", None)),
                    str(getattr(inst, "is_transpose", None)),
                    str(getattr(inst, "perf_mode", None)),
                )
                si = inst.sync_info
                sync_free = si is None or (not si.on_wait and not si.on_update)
                if key == last_key and sync_free:
                    removed += 1
                    continue
                last_key = key
            keep.append(inst)
        bb.instructions[:] = keep
    return removed


def _build(bpc=BPC, n8=N8):
    import concourse.mybir as mybir
    import concourse.tile as tile
    from concourse import bacc
    from contextlib import ExitStack

    BF = mybir.dt.bfloat16
    F32 = mybir.dt.float32
    F8 = mybir.dt.float8e4
    AF = mybir.ActivationFunctionType
    ALU = mybir.AluOpType
    DR = mybir.MatmulPerfMode.DoubleRow

    e8 = 2 * n8  # e-groups handled in fp8
    nc = bacc.Bacc("TRN2", target_bir_lowering=False, debug=False, num_devices=NCORES)
    # x / W1 / W12 arrive host-pre-cast to bf16: halves their DMA traffic and
    # removes all on-device fp32->bf16 casts from the critical path.
    # fc1 contraction split: d rows 0:256 via fp8 DoubleRow (host-cast
    # xT8 = e4m3(x.T/4), W18 = e4m3(4*W1) so the product scale is 1 and the
    # fp8 and bf16 halves accumulate into the same PSUM), rows 256:512 bf16.
    x_d = nc.dram_tensor("xbf", [bpc, N, D], BF, kind="ExternalInput").ap()
    xt8_d = nc.dram_tensor("xT8", [bpc, D // 2, N], F8, kind="ExternalInput").ap()
    xt_d = nc.dram_tensor("xTbf", [bpc, D // 2, N], BF, kind="ExternalInput").ap()
    w18_d = nc.dram_tensor("W18", [D // 2, E], F8, kind="ExternalInput").ap()
    w1_d = nc.dram_tensor("W1bf", [D // 2, E], BF, kind="ExternalInput").ap()
    b1_d = nc.dram_tensor("bias1", [E], F32, kind="ExternalInput").ap()
    b1a_d = nc.dram_tensor("bias1a", [E], F32, kind="ExternalInput").ap()  # (1+a)*b1
    w12_d = nc.dram_tensor("W12bf", [D, D], BF, kind="ExternalInput").ap()  # W1@W2
    c_d = nc.dram_tensor("c", [D], F32, kind="ExternalInput").ap()  # b1@W2 + b2
    out_d = nc.dram_tensor("out", [bpc, N, D], F32, kind="ExternalOutput").ap()

    with tile.TileContext(nc) as tc, ExitStack() as ctx:
        stage = ctx.enter_context(tc.tile_pool(name="stage", bufs=2))
        consts = ctx.enter_context(tc.tile_pool(name="consts", bufs=1))
        xt_p = ctx.enter_context(tc.tile_pool(name="xt", bufs=2))
        projT_p = ctx.enter_context(tc.tile_pool(name="projT", bufs=2))
        keyT_p = ctx.enter_context(tc.tile_pool(name="keyT", bufs=2))
        e_p = ctx.enter_context(tc.tile_pool(name="Et", bufs=2))
        g_p = ctx.enter_context(tc.tile_pool(name="Gt", bufs=2))
        outp = ctx.enter_context(tc.tile_pool(name="outp", bufs=2))
        small = ctx.enter_context(tc.tile_pool(name="small", bufs=2))
        dram = ctx.enter_context(tc.tile_pool(name="dram", bufs=2, space="DRAM"))
        ps = ctx.enter_context(tc.tile_pool(name="ps", bufs=6, space="PSUM"))
        psC = ctx.enter_context(tc.tile_pool(name="psC", bufs=2, space="PSUM"))

        # ---------------- first batch's x in flight before anything else ----
        # xT (host-pre-transposed, fp8 lo-half + bf16 hi-half) first: it gates
        # fc1; then x for the G stage.
        xT8 = xt_p.tile([P, 2, N], F8, tag="xt8")
        nc.sync.dma_start(out=xT8[:], in_=xt8_d[0].rearrange("(g p) n -> p g n", p=P))
        xT = xt_p.tile([P, 2, N], BF, tag="xt")
        nc.sync.dma_start(out=xT[:], in_=xt_d[0].rearrange("(g p) n -> p g n", p=P))
        x_st = stage.tile([P, MG, D], BF, tag="stage")
        nc.sync.dma_start(out=x_st[:], in_=x_d[0].rearrange("(g p) d -> p g d", p=P))

        # ---------------- constants / weights ----------------
        w18 = consts.tile([P, 2, E], F8)
        nc.scalar.dma_start(out=w18[:], in_=w18_d.rearrange("(g p) e -> p g e", p=P))
        w1_bf = consts.tile([P, 2, E], BF)
        w1_src = w1_d.rearrange("(g p) e -> p g e", p=P)
        for dg in range(2):
            nc.scalar.dma_start(out=w1_bf[:, dg, :], in_=w1_src[:, dg, :])

        w12_bf = consts.tile([P, DG, D], BF)
        nc.scalar.dma_start(
            out=w12_bf[:], in_=w12_d.rearrange("(dg p) t -> p dg t", p=P)
        )

        b1t = consts.tile([P, EG], F32)
        nc.gpsimd.dma_start(out=b1t[:], in_=b1_d.rearrange("(g p) -> p g", p=P))
        b1ta = consts.tile([P, EG], F32)
        nc.gpsimd.dma_start(out=b1ta[:], in_=b1a_d.rearrange("(g p) -> p g", p=P))
        ones_sq = consts.tile([P, P], BF)
        nc.vector.memset(ones_sq[:], 1.0)

        # HAM warmup: keep the PE busy while the first batch stages so the
        # clock gate is already at 8/8 when the real matmul stream starts.
        wup = psC.tile([P, 256], F32, tag="psC", name="warmup")
        for _ in range(12):
            nc.tensor.matmul(wup[:, 0:P], ones_sq[:], ones_sq[:, 0:P], start=True, stop=True)
            nc.tensor.matmul(wup[:, P:256], ones_sq[:], ones_sq[:, 0:P], start=True, stop=True)

        # c = b1@W2 + b2 (host-computed input), broadcast to all partitions
        import concourse.bass as bass_mod

        c_bcast = consts.tile([P, D], F32)
        c_src = c_d.rearrange("(o t) -> o t", o=1)
        c_bcast_ap = bass_mod.AP(
            tensor=c_src.tensor,
            offset=c_src.offset,
            ap=[[0, P], c_src.ap[1]],
        )
        nc.gpsimd.dma_start(out=c_bcast[:], in_=c_bcast_ap)

        # ---------------- per-batch pipeline ----------------
        for b in range(bpc):
            if b > 0:
                xT8 = xt_p.tile([P, 2, N], F8, tag="xt8")
                nc.sync.dma_start(
                    out=xT8[:], in_=xt8_d[b].rearrange("(g p) n -> p g n", p=P)
                )
                xT = xt_p.tile([P, 2, N], BF, tag="xt")
                nc.sync.dma_start(
                    out=xT[:], in_=xt_d[b].rearrange("(g p) n -> p g n", p=P)
                )
                x_st = stage.tile([P, MG, D], BF, tag="stage")
                nc.sync.dma_start(
                    out=x_st[:], in_=x_d[b].rearrange("(g p) d -> p g d", p=P)
                )
            x_bf = x_st

            # fc1: projT = W1.T @ xT + b1 ; keyT = relu(projT)
            # first e8 e-groups are written fp8 (scores DoubleRow operands),
            # the rest bf16
            projT8 = projT_p.tile([P, e8, N], F8, name="projT8") if e8 else None
            keyT8 = keyT_p.tile([P, e8, N], F8, name="keyT8") if e8 else None
            nbf = EG - e8
            projTb = projT_p.tile([P, nbf, N], BF, name="projTb") if nbf else None
            keyTb = keyT_p.tile([P, nbf, N], BF, name="keyTb") if nbf else None
            for eg in range(EG):
                pf = [ps.tile([P, 512], F32, tag="ps", name=f"pf{eg}_{j}") for j in range(NJ)]
                for nj in range(NJ):
                    nc.tensor.matmul(
                        pf[nj][:],
                        w18[:, 0:2, eg * P : (eg + 1) * P],
                        xT8[:, 0:2, nj * 512 : (nj + 1) * 512],
                        start=True, stop=False,
                        perf_mode=DR,
                    )
                for dg in range(2):
                    for nj in range(NJ):
                        nc.tensor.matmul(
                            pf[nj][:],
                            w1_bf[:, dg, eg * P : (eg + 1) * P],
                            xT[:, dg, nj * 512 : (nj + 1) * 512],
                            start=False, stop=(dg == 1),
                        )
                if eg < e8:
                    pdst, kdst, ei = projT8, keyT8, eg
                else:
                    pdst, kdst, ei = projTb, keyTb, eg - e8
                for nj in range(NJ):
                    nsl = slice(nj * 512, (nj + 1) * 512)
                    nc.scalar.activation(
                        pdst[:, ei, nsl], pf[nj][:], AF.Identity,
                        bias=b1ta[:, eg : eg + 1], scale=1.0 + DITHER,
                    )
                    nc.vector.tensor_scalar(
                        out=kdst[:, ei, nsl], in0=pf[nj][:],
                        scalar1=b1t[:, eg : eg + 1], scalar2=0.0,
                        op0=ALU.add, op1=ALU.max,
                    )


            # St[m,n] = sum_e keyT[e,m] * projT[e,n];  Et = exp(St/32)
            # fp8 e-group pairs via DoubleRow, remaining e-groups bf16, all
            # accumulating into the same PSUM tile.
            Et = e_p.tile([P, MG, N], BF)
            for mg in range(MG):
                pst = [ps.tile([P, 512], F32, tag="ps", name=f"pst{mg}_{j}") for j in range(NJ)]
                for egp in range(n8):
                    for nj in range(NJ):
                        nc.tensor.matmul(
                            pst[nj][:],
                            keyT8[:, 2 * egp : 2 * egp + 2, mg * P : (mg + 1) * P],
                            projT8[:, 2 * egp : 2 * egp + 2, nj * 512 : (nj + 1) * 512],
                            start=(egp == 0), stop=(egp == n8 - 1 and nbf == 0),
                            perf_mode=DR,
                        )
                for ei in range(nbf):
                    for nj in range(NJ):
                        nc.tensor.matmul(
                            pst[nj][:],
                            keyTb[:, ei, mg * P : (mg + 1) * P],
                            projTb[:, ei, nj * 512 : (nj + 1) * 512],
                            start=(n8 == 0 and ei == 0), stop=(ei == nbf - 1),
                        )
                for nj in range(NJ):
                    nc.scalar.activation(
                        Et[:, mg, nj * 512 : (nj + 1) * 512], pst[nj][:], AF.Exp,
                        bias=0.0, scale=SCALE / (1.0 + DITHER),
                    )

            # rowsum r[n] = sum_m Et[m,n] (all-ones stationary; any psum row = sum)
            r_f32 = small.tile([1, N], F32)
            pr = [ps.tile([P, 512], F32, tag="ps", name=f"pr{j}") for j in range(NJ)]
            for mg in range(MG):
                for nj in range(NJ):
                    nc.tensor.matmul(
                        pr[nj][:], ones_sq[:], Et[:, mg, nj * 512 : (nj + 1) * 512],
                        start=(mg == 0), stop=(mg == MG - 1),
                    )
            for nj in range(NJ):
                nsl = slice(nj * 512, (nj + 1) * 512)
                nc.vector.tensor_copy(r_f32[:, nsl], pr[nj][0:1, :])

            # G[d,n] = sum_m x[m,d] Et[m,n]
            Gt = g_p.tile([P, DG, N], BF)
            for dg in range(DG):
                pg = [ps.tile([P, 512], F32, tag="ps", name=f"pg{dg}_{j}") for j in range(NJ)]
                for mg in range(MG):
                    for nj in range(NJ):
                        nc.tensor.matmul(
                            pg[nj][:],
                            x_bf[:, mg, dg * P : (dg + 1) * P],
                            Et[:, mg, nj * 512 : (nj + 1) * 512],
                            start=(mg == 0), stop=(mg == MG - 1),
                        )
                for nj in range(NJ):
                    nc.vector.tensor_copy(
                        Gt[:, dg, nj * 512 : (nj + 1) * 512], pg[nj][:]
                    )

            # 1/r in [n-partition, 1] layout (bounce through DRAM to transpose)
            r_dram = dram.tile([N], F32)
            nc.sync.dma_start(out=r_dram.rearrange("(o n) -> o n", o=1), in_=r_f32[:1, :])
            rT = small.tile([P, MG], F32)
            nc.sync.dma_start(out=rT[:], in_=r_dram.rearrange("(j p) -> p j", p=P))
            rinv = small.tile([P, MG], F32)
            nc.vector.reciprocal(rinv[:], rT[:])

            # fused fc2: Z[n,t] = sum_d G[d,n] W12[d,t];  out = relu(Z/r + c)
            o_t = outp.tile([P, MG, D], F32)
            for ng in range(MG):
                po = psC.tile([P, D], F32, tag="psC")
                for dg in range(DG):
                    nc.tensor.matmul(
                        po[:],
                        Gt[:, dg, ng * P : (ng + 1) * P],
                        w12_bf[:, dg, :],
                        start=(dg == 0), stop=(dg == DG - 1),
                    )
                if b == bpc - 1:
                    # split the epilogue into half-width chunks so the final
                    # ACT->add->relu->store chain pipelines (shorter tail);
                    # alternate store queues so descriptor issue isn't serial
                    for h in range(2):
                        hsl = slice(h * 256, (h + 1) * 256)
                        osl = o_t[:, ng, hsl]
                        nc.scalar.activation(
                            osl, po[:, hsl], AF.Copy, bias=0.0,
                            scale=rinv[:, ng : ng + 1],
                        )
                        nc.vector.tensor_add(osl, osl, c_bcast[:, hsl])
                        nc.vector.tensor_scalar_max(osl, osl, 0.0)
                        q = nc.gpsimd if h == 0 else nc.sync
                        q.dma_start(
                            out=out_d[b][ng * P : (ng + 1) * P, hsl], in_=osl
                        )
                else:
                    osl = o_t[:, ng, :]
                    nc.scalar.activation(
                        osl, po[:], AF.Copy, bias=0.0, scale=rinv[:, ng : ng + 1]
                    )
                    nc.vector.tensor_add(osl, osl, c_bcast[:])
                    nc.vector.tensor_scalar_max(osl, osl, 0.0)
            if b < bpc - 1:
                nc.gpsimd.dma_start(
                    out=out_d[b].rearrange("(g p) t -> p g t", p=P), in_=o_t[:]
                )

    nc.compile()
    _dedup_ldweights(nc)
    return nc


def get_nc(bpc=BPC, n8=N8):
    if (bpc, n8) not in _CACHE:
        _CACHE[(bpc, n8)] = _build(bpc, n8)
    return _CACHE[(bpc, n8)]


def make_in_maps(x, W1, bias1, W2, bias2):
    import ml_dtypes

    BF = ml_dtypes.bfloat16
    x = np.asarray(x, dtype=np.float32)
    W1 = np.asarray(W1, dtype=np.float32)
    bias1 = np.asarray(bias1, dtype=np.float32)
    W2 = np.asarray(W2, dtype=np.float32)
    bias2 = np.asarray(bias2, dtype=np.float32)
    E4 = ml_dtypes.float8_e4m3
    xbf = np.ascontiguousarray(x.astype(BF))
    xT = x.transpose(0, 2, 1)
    xT8 = np.ascontiguousarray((xT[:, : D // 2, :] / 4.0).astype(E4))
    xTbf = np.ascontiguousarray(xT[:, D // 2 :, :].astype(BF))
    W18 = np.ascontiguousarray((4.0 * W1[: D // 2, :]).astype(E4))
    W1bf = np.ascontiguousarray(W1[D // 2 :, :].astype(BF))
    W12bf = np.ascontiguousarray((W1 @ W2).astype(BF))
    c = (bias1 @ W2 + bias2).astype(np.float32)
    b1a = ((1.0 + DITHER) * bias1).astype(np.float32)
    return [
        {
            "xbf": xbf[i * BPC : (i + 1) * BPC],
            "xT8": xT8[i * BPC : (i + 1) * BPC],
            "xTbf": xTbf[i * BPC : (i + 1) * BPC],
            "W18": W18,
            "W1bf": W1bf,
            "bias1": bias1,
            "bias1a": b1a,
            "W12bf": W12bf,
            "c": c,
        }
        for i in range(NCORES)
    ]


def kernel(x, W1, bias1, W2, bias2):
    from concourse.bass_utils import run_bass_kernel_spmd

    nc = get_nc()
    in_maps = make_in_maps(x, W1, bias1, W2, bias2)
    res = run_bass_kernel_spmd(nc, in_maps, list(range(NCORES)))
    return np.concatenate([res.results[i]["out"] for i in range(NCORES)], axis=0)


# revision 40
# speedup vs baseline: 1.7985x; 1.0031x over previous
"""Bass/Tile TRN2 kernel for nn_AttentionHead (B=64, N=1024, d=512), 8-core data parallel.

Math (per batch):
    proj  = x @ W1 + b1                      [N, 2d]
    S     = proj @ relu(proj).T / sqrt(2d)   [N, N]
    P     = softmax(S, axis=-1)
    F     = P @ proj                         [N, 2d]
    out   = relu(F @ W2 + b2)                [N, d]

Kernel dataflow (transposed-score formulation + fc2 fusion):
    xT    = x.T (DMA transpose)                                 [d, N]
    projT = W1.T @ xT + b1; keyT = relu(projT)   (fp8/bf16)     [2d, N]
    St[m,n] = sum_e keyT[e,m] projT[e,n];  Et = exp(St / 32)    [m, n]
    r[n]  = sum_m Et[m,n]            (ones-column matmul)
    G[d,n] = sum_m x[m,d] Et[m,n]
    out[n,t] = relu( (sum_d G[d,n] W12[d,t]) / r[n] + c[t] )
  where W12 = W1 @ W2 and c = b1 @ W2 + b2 are host-precomputed: since
  P @ proj @ W2 = P@x@(W1 W2) + (P@1) b1 W2 and P rows sum to 1, the whole
  value-path fc1+fc2 collapses into a single [d,d] matmul vs [2d,*] twice.

The scores matmul runs in fp8-e4m3 DoubleRow (2 contraction tiles per
instruction, 2x PE throughput) for the first N8 e-group pairs and bf16 for the
rest, accumulating into the same PSUM bank; fp8 score error is damped by the
1/32 softmax temperature, sim-measured rel_l2 ~1.4e-2 at N8=4. All other
matmuls stay bf16 (value/output-path fp8 error does not average down).
Loops are ordered so each stationary (lhsT) tile serves its free-dim chunks
back-to-back; a post-compile pass (_dedup_ldweights) elides repeat LDWEIGHTS.
"""

import numpy as np

B, N, D = 64, 1024, 512
E = 2 * D
NCORES = 8
BPC = B // NCORES
P = 128
MG = N // P  # 8 token groups
DG = D // P  # 4 d groups
EG = E // P  # 8 e groups
NJ = N // 512  # 2 free-dim chunks
SCALE = float(1.0 / np.sqrt(2.0 * D))
N8 = 4  # e-group PAIRS of the scores contraction done in fp8 DoubleRow (0..4)
DITHER = 3.0 / 64.0  # projT pre-cast scale (1+a): decorrelates its e4m3
# rounding from keyT's (same values post-relu), halving the correlated
# error on the dominant S diagonal; exactly compensated in the exp scale.

_CACHE = {}


def _dedup_ldweights(nc):
    """Delete redundant InstLdweights: consecutive PE weight-loads of the same
    SBUF region keep the PE array's stationary operand, so the repeat load is a
    no-op costing ~107ns. Only sync-free LDWs are removed (waits/updates were
    already hoisted by bacc's move_matmul_waits_to_ldweights)."""
    import concourse.mybir as mybir

    removed = 0
    for bb in nc.m.functions[0].blocks:
        last_key = None
        keep = []
        for inst in bb.instructions:
            if str(getattr(inst, "engine", "")) != "EngineType.PE":
                keep.append(inst)
                continue
            if isinstance(inst, mybir.InstLdweights):
                ap = inst.ins[0]
                key = (
                    getattr(ap, "memref", None),
                    getattr(ap, "offset", None),
                    str(getattr(ap, "ap", None)),
                    str(getattr(ap, "dtype", None)),
                    str(getattr(inst, "tile_position", None)),
                    str(getattr(inst, "is_transpose", None)),
                    str(getattr(inst, "perf_mode", None)),
                )
                si = inst.sync_info
                sync_free = si is None or (not si.on_wait and not si.on_update)
                if key == last_key and sync_free:
                    removed += 1
                    continue
                last_key = key
            keep.append(inst)
        bb.instructions[:] = keep
    return removed


def _build(bpc=BPC, n8=N8):
    import concourse.mybir as mybir
    import concourse.tile as tile
    from concourse import bacc
    from contextlib import ExitStack

    BF = mybir.dt.bfloat16
    F32 = mybir.dt.float32
    F8 = mybir.dt.float8e4
    AF = mybir.ActivationFunctionType
    ALU = mybir.AluOpType
    DR = mybir.MatmulPerfMode.DoubleRow

    e8 = 2 * n8  # e-groups handled in fp8
    nc = bacc.Bacc("TRN2", target_bir_lowering=False, debug=False, num_devices=NCORES)
    # x / W1 / W12 arrive host-pre-cast to bf16: halves their DMA traffic and
    # removes all on-device fp32->bf16 casts from the critical path.
    # fc1 contraction split: d rows 0:256 via fp8 DoubleRow (host-cast
    # xT8 = e4m3(x.T/4), W18 = e4m3(4*W1) so the product scale is 1 and the
    # fp8 and bf16 halves accumulate into the same PSUM), rows 256:512 bf16.
    x_d = nc.dram_tensor("xbf", [bpc, N, D], BF, kind="ExternalInput").ap()
    xt8_d = nc.dram_tensor("xT8", [bpc, D // 2, N], F8, kind="ExternalInput").ap()
    xt_d = nc.dram_tensor("xTbf", [bpc, D // 2, N], BF, kind="ExternalInput").ap()
    w18_d = nc.dram_tensor("W18", [D // 2, E], F8, kind="ExternalInput").ap()
    w1_d = nc.dram_tensor("W1bf", [D // 2, E], BF, kind="ExternalInput").ap()
    b1_d = nc.dram_tensor("bias1", [E], F32, kind="ExternalInput").ap()
    b1a_d = nc.dram_tensor("bias1a", [E], F32, kind="ExternalInput").ap()  # (1+a)*b1
    w12_d = nc.dram_tensor("W12bf", [D, D], BF, kind="ExternalInput").ap()  # W1@W2
    c_d = nc.dram_tensor("c", [D], F32, kind="ExternalInput").ap()  # b1@W2 + b2
    out_d = nc.dram_tensor("out", [bpc, N, D], F32, kind="ExternalOutput").ap()

    with tile.TileContext(nc) as tc, ExitStack() as ctx:
        stage = ctx.enter_context(tc.tile_pool(name="stage", bufs=2))
        consts = ctx.enter_context(tc.tile_pool(name="consts", bufs=1))
        xt_p = ctx.enter_context(tc.tile_pool(name="xt", bufs=2))
        projT_p = ctx.enter_context(tc.tile_pool(name="projT", bufs=2))
        keyT_p = ctx.enter_context(tc.tile_pool(name="keyT", bufs=2))
        e_p = ctx.enter_context(tc.tile_pool(name="Et", bufs=2))
        g_p = ctx.enter_context(tc.tile_pool(name="Gt", bufs=2))
        outp = ctx.enter_context(tc.tile_pool(name="outp", bufs=2))
        small = ctx.enter_context(tc.tile_pool(name="small", bufs=2))
        dram = ctx.enter_context(tc.tile_pool(name="dram", bufs=2, space="DRAM"))
        ps = ctx.enter_context(tc.tile_pool(name="ps", bufs=6, space="PSUM"))
        psC = ctx.enter_context(tc.tile_pool(name="psC", bufs=2, space="PSUM"))

        # ---------------- first batch's x in flight before anything else ----
        # xT (host-pre-transposed, fp8 lo-half + bf16 hi-half) first: it gates
        # fc1; then x for the G stage.
        xT8 = xt_p.tile([P, 2, N], F8, tag="xt8")
        xt80_src = xt8_d[0].rearrange("(g p) n -> p g n", p=P)
        xT = xt_p.tile([P, 2, N], BF, tag="xt")
        xt0_src = xt_d[0].rearrange("(g p) n -> p g n", p=P)
        for nj in range(NJ):
            nsl = slice(nj * 512, (nj + 1) * 512)
            nc.sync.dma_start(out=xT8[:, :, nsl], in_=xt80_src[:, :, nsl])
            nc.sync.dma_start(out=xT[:, :, nsl], in_=xt0_src[:, :, nsl])
        x_st = stage.tile([P, MG, D], BF, tag="stage")
        nc.sync.dma_start(out=x_st[:], in_=x_d[0].rearrange("(g p) d -> p g d", p=P))

        # ---------------- constants / weights ----------------
        w18 = consts.tile([P, 2, E], F8)
        nc.scalar.dma_start(out=w18[:], in_=w18_d.rearrange("(g p) e -> p g e", p=P))
        w1_bf = consts.tile([P, 2, E], BF)
        w1_src = w1_d.rearrange("(g p) e -> p g e", p=P)
        for dg in range(2):
            nc.scalar.dma_start(out=w1_bf[:, dg, :], in_=w1_src[:, dg, :])

        w12_bf = consts.tile([P, DG, D], BF)
        nc.scalar.dma_start(
            out=w12_bf[:], in_=w12_d.rearrange("(dg p) t -> p dg t", p=P)
        )

        b1t = consts.tile([P, EG], F32)
        nc.gpsimd.dma_start(out=b1t[:], in_=b1_d.rearrange("(g p) -> p g", p=P))
        b1ta = consts.tile([P, EG], F32)
        nc.gpsimd.dma_start(out=b1ta[:], in_=b1a_d.rearrange("(g p) -> p g", p=P))
        ones_sq = consts.tile([P, P], BF)
        nc.vector.memset(ones_sq[:], 1.0)

        # HAM warmup: keep the PE busy while the first batch stages so the
        # clock gate is already at 8/8 when the real matmul stream starts.
        wup = psC.tile([P, 256], F32, tag="psC", name="warmup")
        for _ in range(12):
            nc.tensor.matmul(wup[:, 0:P], ones_sq[:], ones_sq[:, 0:P], start=True, stop=True)
            nc.tensor.matmul(wup[:, P:256], ones_sq[:], ones_sq[:, 0:P], start=True, stop=True)

        # c = b1@W2 + b2 (host-computed input), broadcast to all partitions
        import concourse.bass as bass_mod

        c_bcast = consts.tile([P, D], F32)
        c_src = c_d.rearrange("(o t) -> o t", o=1)
        c_bcast_ap = bass_mod.AP(
            tensor=c_src.tensor,
            offset=c_src.offset,
            ap=[[0, P], c_src.ap[1]],
        )
        nc.gpsimd.dma_start(out=c_bcast[:], in_=c_bcast_ap)

        # ---------------- per-batch pipeline ----------------
        for b in range(bpc):
            if b > 0:
                xT8 = xt_p.tile([P, 2, N], F8, tag="xt8")
                nc.sync.dma_start(
                    out=xT8[:], in_=xt8_d[b].rearrange("(g p) n -> p g n", p=P)
                )
                xT = xt_p.tile([P, 2, N], BF, tag="xt")
                nc.sync.dma_start(
                    out=xT[:], in_=xt_d[b].rearrange("(g p) n -> p g n", p=P)
                )
                x_st = stage.tile([P, MG, D], BF, tag="stage")
                nc.sync.dma_start(
                    out=x_st[:], in_=x_d[b].rearrange("(g p) d -> p g d", p=P)
                )
            x_bf = x_st

            # fc1: projT = W1.T @ xT + b1 ; keyT = relu(projT)
            # first e8 e-groups are written fp8 (scores DoubleRow operands),
            # the rest bf16
            projT8 = projT_p.tile([P, e8, N], F8, name="projT8") if e8 else None
            keyT8 = keyT_p.tile([P, e8, N], F8, name="keyT8") if e8 else None
            nbf = EG - e8
            projTb = projT_p.tile([P, nbf, N], BF, name="projTb") if nbf else None
            keyTb = keyT_p.tile([P, nbf, N], BF, name="keyTb") if nbf else None
            for eg in range(EG):
                pf = [ps.tile([P, 512], F32, tag="ps", name=f"pf{eg}_{j}") for j in range(NJ)]
                for nj in range(NJ):
                    nc.tensor.matmul(
                        pf[nj][:],
                        w18[:, 0:2, eg * P : (eg + 1) * P],
                        xT8[:, 0:2, nj * 512 : (nj + 1) * 512],
                        start=True, stop=False,
                        perf_mode=DR,
                    )
                for dg in range(2):
                    for nj in range(NJ):
                        nc.tensor.matmul(
                            pf[nj][:],
                            w1_bf[:, dg, eg * P : (eg + 1) * P],
                            xT[:, dg, nj * 512 : (nj + 1) * 512],
                            start=False, stop=(dg == 1),
                        )
                if eg < e8:
                    pdst, kdst, ei = projT8, keyT8, eg
                else:
                    pdst, kdst, ei = projTb, keyTb, eg - e8
                for nj in range(NJ):
                    nsl = slice(nj * 512, (nj + 1) * 512)
                    nc.scalar.activation(
                        pdst[:, ei, nsl], pf[nj][:], AF.Identity,
                        bias=b1ta[:, eg : eg + 1], scale=1.0 + DITHER,
                    )
                    nc.vector.tensor_scalar(
                        out=kdst[:, ei, nsl], in0=pf[nj][:],
                        scalar1=b1t[:, eg : eg + 1], scalar2=0.0,
                        op0=ALU.add, op1=ALU.max,
                    )


            # St[m,n] = sum_e keyT[e,m] * projT[e,n];  Et = exp(St/32)
            # fp8 e-group pairs via DoubleRow, remaining e-groups bf16, all
            # accumulating into the same PSUM tile.
            Et = e_p.tile([P, MG, N], BF)
            for mg in range(MG):
                pst = [ps.tile([P, 512], F32, tag="ps", name=f"pst{mg}_{j}") for j in range(NJ)]
                for egp in range(n8):
                    for nj in range(NJ):
                        nc.tensor.matmul(
                            pst[nj][:],
                            keyT8[:, 2 * egp : 2 * egp + 2, mg * P : (mg + 1) * P],
                            projT8[:, 2 * egp : 2 * egp + 2, nj * 512 : (nj + 1) * 512],
                            start=(egp == 0), stop=(egp == n8 - 1 and nbf == 0),
                            perf_mode=DR,
                        )
                for ei in range(nbf):
                    for nj in range(NJ):
                        nc.tensor.matmul(
                            pst[nj][:],
                            keyTb[:, ei, mg * P : (mg + 1) * P],
                            projTb[:, ei, nj * 512 : (nj + 1) * 512],
                            start=(n8 == 0 and ei == 0), stop=(ei == nbf - 1),
                        )
                for nj in range(NJ):
                    nc.scalar.activation(
                        Et[:, mg, nj * 512 : (nj + 1) * 512], pst[nj][:], AF.Exp,
                        bias=0.0, scale=SCALE / (1.0 + DITHER),
                    )

            # rowsum r[n] = sum_m Et[m,n] (all-ones stationary; any psum row = sum)
            r_f32 = small.tile([1, N], F32)
            pr = [ps.tile([P, 512], F32, tag="ps", name=f"pr{j}") for j in range(NJ)]
            for mg in range(MG):
                for nj in range(NJ):
                    nc.tensor.matmul(
                        pr[nj][:], ones_sq[:], Et[:, mg, nj * 512 : (nj + 1) * 512],
                        start=(mg == 0), stop=(mg == MG - 1),
                    )
            for nj in range(NJ):
                nsl = slice(nj * 512, (nj + 1) * 512)
                nc.vector.tensor_copy(r_f32[:, nsl], pr[nj][0:1, :])

            # G[d,n] = sum_m x[m,d] Et[m,n]
            Gt = g_p.tile([P, DG, N], BF)
            for dg in range(DG):
                pg = [ps.tile([P, 512], F32, tag="ps", name=f"pg{dg}_{j}") for j in range(NJ)]
                for mg in range(MG):
                    for nj in range(NJ):
                        nc.tensor.matmul(
                            pg[nj][:],
                            x_bf[:, mg, dg * P : (dg + 1) * P],
                            Et[:, mg, nj * 512 : (nj + 1) * 512],
                            start=(mg == 0), stop=(mg == MG - 1),
                        )
                for nj in range(NJ):
                    nc.vector.tensor_copy(
                        Gt[:, dg, nj * 512 : (nj + 1) * 512], pg[nj][:]
                    )

            # 1/r in [n-partition, 1] layout (bounce through DRAM to transpose)
            r_dram = dram.tile([N], F32)
            nc.sync.dma_start(out=r_dram.rearrange("(o n) -> o n", o=1), in_=r_f32[:1, :])
            rT = small.tile([P, MG], F32)
            nc.sync.dma_start(out=rT[:], in_=r_dram.rearrange("(j p) -> p j", p=P))
            rinv = small.tile([P, MG], F32)
            nc.vector.reciprocal(rinv[:], rT[:])

            # fused fc2: Z[n,t] = sum_d G[d,n] W12[d,t];  out = relu(Z/r + c)
            o_t = outp.tile([P, MG, D], F32)
            for ng in range(MG):
                po = psC.tile([P, D], F32, tag="psC")
                for dg in range(DG):
                    nc.tensor.matmul(
                        po[:],
                        Gt[:, dg, ng * P : (ng + 1) * P],
                        w12_bf[:, dg, :],
                        start=(dg == 0), stop=(dg == DG - 1),
                    )
                osl = o_t[:, ng, :]
                nc.scalar.activation(
                    osl, po[:], AF.Copy, bias=0.0, scale=rinv[:, ng : ng + 1]
                )
                nc.vector.tensor_add(osl, osl, c_bcast[:])
                nc.vector.tensor_scalar_max(osl, osl, 0.0)
                if b == bpc - 1:
                    # store each ng as it completes, alternating queues so
                    # descriptor issue doesn't serialize the tail
                    q = nc.gpsimd if ng % 2 == 0 else nc.sync
                    q.dma_start(out=out_d[b][ng * P : (ng + 1) * P, :], in_=osl)
            if b < bpc - 1:
                nc.gpsimd.dma_start(
                    out=out_d[b].rearrange("(g p) t -> p g t", p=P), in_=o_t[:]
                )

    nc.compile()
    _dedup_ldweights(nc)
    return nc


def get_nc(bpc=BPC, n8=N8):
    if (bpc, n8) not in _CACHE:
        _CACHE[(bpc, n8)] = _build(bpc, n8)
    return _CACHE[(bpc, n8)]


def make_in_maps(x, W1, bias1, W2, bias2):
    import ml_dtypes

    BF = ml_dtypes.bfloat16
    x = np.asarray(x, dtype=np.float32)
    W1 = np.asarray(W1, dtype=np.float32)
    bias1 = np.asarray(bias1, dtype=np.float32)
    W2 = np.asarray(W2, dtype=np.float32)
    bias2 = np.asarray(bias2, dtype=np.float32)
    E4 = ml_dtypes.float8_e4m3
    xbf = np.ascontiguousarray(x.astype(BF))
    xT = x.transpose(0, 2, 1)
    xT8 = np.ascontiguousarray((xT[:, : D // 2, :] / 4.0).astype(E4))
    xTbf = np.ascontiguousarray(xT[:, D // 2 :, :].astype(BF))
    W18 = np.ascontiguousarray((4.0 * W1[: D // 2, :]).astype(E4))
    W1bf = np.ascontiguousarray(W1[D // 2 :, :].astype(BF))
    W12bf = np.ascontiguousarray((W1 @ W2).astype(BF))
    c = (bias1 @ W2 + bias2).astype(np.float32)
    b1a = ((1.0 + DITHER) * bias1).astype(np.float32)
    return [
        {
            "xbf": xbf[i * BPC : (i + 1) * BPC],
            "xT8": xT8[i * BPC : (i + 1) * BPC],
            "xTbf": xTbf[i * BPC : (i + 1) * BPC],
            "W18": W18,
            "W1bf": W1bf,
            "bias1": bias1,
            "bias1a": b1a,
            "W12bf": W12bf,
            "c": c,
        }
        for i in range(NCORES)
    ]


def kernel(x, W1, bias1, W2, bias2):
    from concourse.bass_utils import run_bass_kernel_spmd

    nc = get_nc()
    in_maps = make_in_maps(x, W1, bias1, W2, bias2)
    res = run_bass_kernel_spmd(nc, in_maps, list(range(NCORES)))
    return np.concatenate([res.results[i]["out"] for i in range(NCORES)], axis=0)
